# revision 50
# baseline (speedup 1.0000x reference)
"""Trainium2 Bass kernel: 3-layer GAT message passing, 8-core SPMD.

Dst-sharded edge phase (dma_gather + one-hot scatter matmuls into PSUM;
self-loop rows bulk-loaded, bypassing gather descriptors), node-sharded
table build interleaved into the previous edge phase, split AllGathers.
"""
"""GAT message-passing kernel for TRN2, 8-core SPMD.

Per layer:
  node phase (node-sharded): table rows [h' | alpha_s | alpha_d | pad] for own
  nodes via PE matmuls with host-packed rhs [W | U_s | U_d].
  AllGather table -> full gather table in every core's DRAM.
  edge phase (dst-sharded): per dst-window of 128 nodes, dma_gather rows for
  the window's edges (src-indexed, int16 lo/hi split), build one-hot
  scatter/expand matrices on DVE, expand alpha_d via PE matmul,
  w = max(exp(e), exp(slope*e)) == exp(leaky_relu(e)), weighted messages
  scatter-accumulated into PSUM via one-hot matmul, normalized by the
  softmax denominator (segment max subtraction dropped; shift-invariant).
"""

import math
from dataclasses import dataclass, field

import numpy as np
import ml_dtypes

import concourse.bass as bass
import concourse.bacc as bacc
import concourse.mybir as mybir
import concourse.tile as tile
from concourse import library_config

F32 = mybir.dt.float32
BF16 = mybir.dt.bfloat16
I16 = mybir.dt.int16
AF = mybir.ActivationFunctionType
ALU = mybir.AluOpType
NEG_SLOPE = 0.2

bf = ml_dtypes.bfloat16


@dataclass
class Cfg:
    N: int = 50000
    n_cores: int = 8
    IN: int = 256
    HID: int = 128
    OUT: int = 128
    TSPLIT: int = 32768
    K_FUSE: int = 8
    NQ: int = 4  # SWDGE queues: round-robin descgen across Q7 core pairs

    @property
    def n_loc(self):
        return self.N // self.n_cores

    @property
    def n_win(self):
        return (self.n_loc + 127) // 128

    def win_size(self, t):
        return min(128, self.n_loc - t * 128)

    @property
    def WA(self):
        return 26

    @property
    def nA(self):
        return min(self.WA * 128, self.n_loc)

    @property
    def nB(self):
        return self.n_loc - self.nA

    # table pieces: each table's AllGather is split so the LAST piece is
    # small -- its wire time is the only part not hidden behind the
    # producer's remaining window projections (gathers wait on ALL pieces,
    # but earlier pieces finish during later writes).
    @property
    def WA1(self):
        return 13          # windows in piece A1 (A2 = WA - WA1)

    @property
    def WB1(self):
        return 17          # windows in piece B1 (B2 = rest)

    @property
    def nA1(self):
        return self.WA1 * 128

    @property
    def nA2(self):
        return self.nA - self.nA1

    @property
    def nB1(self):
        return self.WB1 * 128

    @property
    def nB2(self):
        return self.nB - self.nB1

    @property
    def layers(self):
        hid, out = self.HID, self.OUT
        ls = []
        for (H, C, in_ch) in ((4, hid // 2, hid), (2, out, 2 * hid), (1, out, out)):
            HC = H * C
            row = HC + 2 * H
            row_pad = ((row * 2 + 255) // 256) * 256 // 2
            ls.append(dict(H=H, C=C, in_ch=in_ch, HC=HC, row=row_pad,
                           as_off=HC, ad_off=HC + H))
        return ls


# ---------------------------------------------------------------- host plan

@dataclass
class Plan:
    T_lo: list
    T_hi: list
    idx_lo: list
    idx_hi: list
    dstloc_col: list
    dstloc_row: list
    NT: int = 0
    cols_lo: int = 0
    cols_hi: int = 0
    win_tile_off: list = field(default_factory=list)
    win_lo_off: list = field(default_factory=list)
    win_hi_off: list = field(default_factory=list)


def plan_edges(edge_index, cfg: Cfg) -> Plan:
    """Balanced plan: per core, pack local dst nodes into windows so that
    per-window (lo, hi) edge sums are near-uniform across windows AND cores
    (kills ceil + max-over-core tile padding).  Nodes with original local
    index < nA go to windows [0, WA) (table A), the rest to [WA, W) (B) --
    membership is decided by original index so lo/hi degrees are fixed
    before packing (no circularity).  perm[c][p] = original local node at
    permuted position p; inputs (xT) are host-permuted, outputs unpermuted
    in kernel()."""
    N, C = cfg.N, cfg.n_cores
    n_loc = cfg.n_loc
    W = cfg.n_win
    src = np.asarray(edge_index[0], np.int64)      # self loops handled by a
    dst = np.asarray(edge_index[1], np.int64)      # direct per-window load
    s_core = src // n_loc
    s_loc = src % n_loc
    lo_edge = s_loc < cfg.nA                       # src in table A?
    d_core = dst // n_loc
    d_loc = dst % n_loc

    # per-node lo/hi in-degree
    lo_deg = np.zeros(N, np.int64)
    hi_deg = np.zeros(N, np.int64)
    np.add.at(lo_deg, dst[lo_edge], 1)
    np.add.at(hi_deg, dst[~lo_edge], 1)

    # per-core packing
    perm = []        # perm[c][p] = original local idx at permuted pos p
    win_of = np.zeros(N, np.int64)
    slot_of = np.zeros(N, np.int64)
    for c in range(C):
        ld = lo_deg[c * n_loc:(c + 1) * n_loc]
        hd = hi_deg[c * n_loc:(c + 1) * n_loc]
        pc = np.zeros(n_loc, np.int64)
        import os
        identity = bool(int(os.environ.get("PLAN_IDENTITY", "0")))
        for half, (wins, nodes) in enumerate((
                (np.arange(cfg.WA), np.arange(cfg.nA)),
                (np.arange(cfg.WA, W), np.arange(cfg.nA, n_loc)))):
            caps = np.array([cfg.win_size(w) for w in wins], np.int64)
            lt = np.zeros(len(wins))
            ht = np.zeros(len(wins))
            space = caps.copy()
            # per-window targets proportional to window size
            lo_t = ld[nodes].sum() * caps / caps.sum()
            hi_t = hd[nodes].sum() * caps / caps.sum()
            order = (nodes if identity else
                     nodes[np.argsort(-(ld[nodes] + hd[nodes]), kind="stable")])
            fill_ptr = 0
            for nloc_i in order:
                l, h = ld[nloc_i], hd[nloc_i]
                if identity:
                    if space[fill_ptr] <= 0:
                        fill_ptr += 1
                    wi = fill_ptr
                else:
                    score = np.maximum((lt + l) - lo_t, (ht + h) - hi_t)
                    score[space <= 0] = 1 << 30
                    wi = int(np.argmin(score))
                lt[wi] += l
                ht[wi] += h
                space[wi] -= 1
                w = wins[wi]
                slot = cfg.win_size(w) - space[wi] - 1
                win_of[c * n_loc + nloc_i] = w
                slot_of[c * n_loc + nloc_i] = slot
                pc[(w * 128 if w < cfg.WA else
                    cfg.nA + (w - cfg.WA) * 128) + slot] = nloc_i
        perm.append(pc)

    # permuted position of each node within its owner's table
    pos = np.zeros(N, np.int64)
    for c in range(C):
        pc = perm[c]
        pos[c * n_loc + pc] = np.arange(n_loc)
    idx_a = s_core * cfg.nA + pos[src]             # valid when lo_edge
    idx_b = s_core * cfg.nB + (pos[src] - cfg.nA)  # valid when ~lo_edge

    win = win_of[dst]
    dloc = slot_of[dst]
    core = d_core

    buckets = {}
    for c in range(C):
        m_c = core == c
        for t in range(W):
            m = m_c & (win == t)
            ml = m & lo_edge
            mh = m & ~lo_edge
            sl, dll = idx_a[ml], dloc[ml]
            o = np.argsort(sl, kind="stable")
            buckets[(c, t, 0)] = (sl[o], dll[o])
            sh, dlh = idx_b[mh], dloc[mh]
            o = np.argsort(sh, kind="stable")
            buckets[(c, t, 1)] = (sh[o], dlh[o])

    T_lo = [max(math.ceil(len(buckets[(c, t, 0)][0]) / 128) for c in range(C))
            for t in range(W)]
    T_hi = [max(math.ceil(len(buckets[(c, t, 1)][0]) / 128) for c in range(C))
            for t in range(W)]

    NT = sum(T_lo) + sum(T_hi) + W  # +1 self tile per window
    p = Plan(T_lo=T_lo, T_hi=T_hi, idx_lo=[], idx_hi=[], dstloc_col=[],
             dstloc_row=[], NT=NT, cols_lo=max(sum(T_lo) * 8, 8),
             cols_hi=max(sum(T_hi) * 8, 8))
    p.perm = perm
    p.blen = [dict() for _ in range(C)]   # (t, half) -> actual edge count
    p.call_order = []                     # (t, half, q0, q) per gather call
    off = olo = ohi = 0
    for t in range(W):
        p.win_tile_off.append(off)
        p.win_lo_off.append(olo)
        p.win_hi_off.append(ohi)
        off += T_lo[t] + T_hi[t] + 1
        olo += T_lo[t] * 8
        ohi += T_hi[t] * 8

    for c in range(C):
        ilo = np.zeros((128, p.cols_lo), np.int16)
        ihi = np.zeros((128, p.cols_hi), np.int16)
        dcol = np.full((128, NT), -1.0, np.float32)
        for t in range(W):
            for half, (idx_arr, col_off, Tn) in enumerate(
                ((ilo, p.win_lo_off[t], T_lo[t]),
                 (ihi, p.win_hi_off[t], T_hi[t]))):
                if Tn == 0:
                    continue
                s_, dl = buckets[(c, t, half)]
                n = Tn * 128
                # trailing -1 idxs: the Q7 ucode truncates them at runtime;
                # num_idxs_reg carries the per-core actual count so the
                # decode-side ring booking stays consistent (a mismatch
                # deadlocks the SWDGE ring).
                import os as _os
                if _os.environ.get("TRUNC_OFF"):
                    si = np.zeros(n, np.int64)
                else:
                    si = np.full(n, -1, np.int64)
                si[:len(s_)] = s_
                p.blen[c][(t, half)] = len(s_)
                dli = np.full(n, -1.0, np.float32)
                dli[:len(dl)] = dl
                blk = si.astype(np.int16).reshape(Tn * 8, 16).T
                idx_arr[:, col_off:col_off + Tn * 8] = np.tile(blk, (8, 1))
                tb = p.win_tile_off[t] + (0 if half == 0 else T_lo[t])
                dcol[:, tb:tb + Tn] = dli.reshape(Tn, 128).T
            ts = p.win_tile_off[t] + T_lo[t] + T_hi[t]
            nn_w = min(128, n_loc - t * 128)
            selfcol = np.full(128, -1.0, np.float32)
            selfcol[:nn_w] = np.arange(nn_w)
            dcol[:, ts] = selfcol
        p.idx_lo.append(ilo)
        p.idx_hi.append(ihi)
        p.dstloc_col.append(dcol.astype(bf))
        p.dstloc_row.append(dcol.T.reshape(1, NT * 128).astype(bf))
    return p


# ------------------------------------------------------------- host weights

def prep_weights(inp, cfg: Cfg):
    """Pack per-layer projection weights.  h' columns are stored
    (c, h)-interleaved (head innermost) so the per-head message weighting
    broadcasts on a MIDDLE dim -- DVE runs 2x faster than with an
    innermost-dim broadcast."""
    out = {}
    for li, (wk, ak, dk) in enumerate(
            (("g1_W", "g1_as", "g1_ad"), ("g2_W", "g2_as", "g2_ad"),
             ("g3_W", "g3_as", "g3_ad"))):
        L = cfg.layers[li]
        Wm = np.asarray(inp[wk], np.float32)
        if li == 1:
            # layer-1 input = layer-0 output, which we emit (c,h)-interleaved
            L0 = cfg.layers[0]
            j = np.arange(L0["HC"])
            perm_in = (j % L0["H"]) * L0["C"] + j // L0["H"]
            Wm = Wm[perm_in]
        a_s = np.asarray(inp[ak], np.float32)
        a_d = np.asarray(inp[dk], np.float32)
        H, Cc = L["H"], L["C"]
        U_s = np.stack([Wm[:, h * Cc:(h + 1) * Cc] @ a_s[h] for h in range(H)], 1)
        U_d = np.stack([Wm[:, h * Cc:(h + 1) * Cc] @ a_d[h] for h in range(H)], 1)
        # (h,c) -> (c,h) column interleave
        Wm_i = np.ascontiguousarray(
            Wm.reshape(-1, H, Cc).transpose(0, 2, 1).reshape(Wm.shape))
        out[f"WG{li+1}"] = np.concatenate([Wm_i, U_s, U_d], 1).astype(bf)
        bias = np.asarray(inp[f"g{li+1}_b"], np.float32)
        if li == 0:
            bias = np.ascontiguousarray(
                bias.reshape(H, Cc).T.reshape(-1))
        out[f"bG{li+1}"] = np.ascontiguousarray(np.broadcast_to(
            bias[None, :], (128, len(bias))))
    out["Wm1"] = np.asarray(inp["W1"], np.float32).astype(bf)
    out["Wm2"] = np.asarray(inp["W2"], np.float32).astype(bf)
    out["b1c"] = np.ascontiguousarray(np.asarray(inp["b1"], np.float32)[:, None])
    out["b2c"] = np.ascontiguousarray(np.asarray(inp["b2"], np.float32)[:, None])
    out["iota_row"] = np.ascontiguousarray(
        np.broadcast_to(np.arange(128, dtype=np.float32), (128, 128))).astype(bf)
    out["iota_col"] = np.arange(128, dtype=np.float32)[:, None].astype(bf)
    out["ident"] = np.eye(128, dtype=np.float32).astype(bf)
    return out


# ---------------------------------------------------------------- builder

def build(nc, cfg: Cfg, p: Plan, dbg=False):
    W = cfg.n_win
    n_loc = cfg.n_loc
    Ls = cfg.layers
    MAXT = max(p.T_lo[t] + p.T_hi[t] for t in range(W)) + 1
    MAXHC = max(L["HC"] for L in Ls)
    MAXH = max(L["H"] for L in Ls)

    def din(name, shape, dt):
        return nc.dram_tensor(name, list(shape), dt, kind="ExternalInput")

    xT = din("xT", (cfg.IN, n_loc), BF16)
    Wm1 = din("Wm1", (cfg.IN, cfg.HID), BF16)
    Wm2 = din("Wm2", (cfg.HID, cfg.HID), BF16)
    b1c = din("b1c", (cfg.HID, 1), F32)
    b2c = din("b2c", (cfg.HID, 1), F32)
    WG = [din(f"WG{i+1}", (Ls[i]["in_ch"], Ls[i]["HC"] + 2 * Ls[i]["H"]), BF16)
          for i in range(3)]
    bG = [din(f"bG{i+1}", (128, Ls[i]["HC"] if i == 0 else Ls[i]["C"]), F32)
          for i in range(3)]
    iota_row_d = din("iota_row", (128, 128), BF16)
    iota_col_d = din("iota_col", (128, 1), BF16)
    ident_d = din("ident", (128, 128), BF16)
    idx_lo_d = din("idx_lo", (128, p.cols_lo), I16)
    idx_hi_d = din("idx_hi", (128, p.cols_hi), I16)
    dstcol_d = din("dstcol", (128, p.NT), BF16)
    ncall = 3 * sum(math.ceil(p.T_lo[t] / 8) + math.ceil(p.T_hi[t] / 8)
                    for t in range(W))
    gcnt_d = din("gcnt", (1, ncall), mybir.dt.uint32)
    out_d = nc.dram_tensor("out", [n_loc, cfg.OUT], F32, kind="ExternalOutput")
    dbg_d = {}
    if dbg:
        L0 = Ls[0]
        T0 = p.T_lo[0] + p.T_hi[0]
        for nm, shape in (("d_piece0", (cfg.HID, n_loc)),
                          ("d_tin0", (n_loc, L0["row"])),
                          ("d_tag0", (cfg.N, L0["row"])),
                          ("d_g0", (128, T0 * L0["row"])),
                          ("d_oT0", (128, T0 * 128)),
                          ("d_oD0", (128, T0 * 128)),
                          ("d_esb0", (128, T0 * L0["H"])),
                          ("d_msg0", (128, T0 * (L0["HC"] + L0["H"]))),
                          ("d_psw0", (128, L0["HC"] + L0["H"])),
                          ("d_ad0", (128, L0["H"]))):
            dt = BF16 if nm in ("d_tin0", "d_tag0", "d_g0", "d_oT0", "d_oD0",
                                "d_msg0", "d_piece0", "d_ad0") else F32
            dbg_d[nm] = nc.dram_tensor(nm, list(shape), dt, kind="ExternalOutput")

    rep = [list(range(cfg.n_cores))]

    with tile.TileContext(nc) as tc:
        with (
            tc.tile_pool(name="const", bufs=1) as cpool,
            tc.tile_pool(name="glo", bufs=getattr(cfg, "lo_bufs", 12)) as glopool,
            tc.tile_pool(name="ghi", bufs=getattr(cfg, "hi_bufs", 3)) as ghipool,
            tc.tile_pool(name="oh", bufs=2) as opool,
            tc.tile_pool(name="msg", bufs=2) as mpool,
            tc.tile_pool(name="small", bufs=4) as spool,
            tc.tile_pool(name="nodes", bufs=3) as npool,
            tc.tile_pool(name="psA", bufs=getattr(cfg, "psA_bufs", 2), space="PSUM") as psA,
            tc.tile_pool(name="psB", bufs=getattr(cfg, "psB_bufs", 2), space="PSUM") as psB,
            tc.tile_pool(name="psC", bufs=2, space="PSUM") as psC,
            tc.tile_pool(name="dram", bufs=1, space="DRAM") as dpool,
        ):
            nc.gpsimd.load_library(library_config.mlp)

            def load_const(handle, shape, dtp, tag):
                t = cpool.tile(list(shape), dtp, tag=tag, name=tag)
                nc.sync.dma_start(t[:], handle[:])
                return t

            iota_row = load_const(iota_row_d, (128, 128), BF16, "iota_row")
            iota_col = load_const(iota_col_d, (128, 1), BF16, "iota_col")
            ident = load_const(ident_d, (128, 128), BF16, "ident")
            idx_lo = load_const(idx_lo_d, (128, p.cols_lo), I16, "idx_lo")
            idx_hi = load_const(idx_hi_d, (128, p.cols_hi), I16, "idx_hi")
            dstcol = load_const(dstcol_d, (128, p.NT), BF16, "dstcol")
            gcnt = load_const(gcnt_d, (1, ncall), mybir.dt.uint32, "gcnt")
            cnt_reg = nc.gpsimd.alloc_register("gcnt_reg")
            wg_sb = []
            for i in range(3):
                L = Ls[i]
                kch = L["in_ch"] // 128
                t = cpool.tile([128, kch, L["HC"] + 2 * L["H"]], BF16, tag=f"wg{i}",
                               name=f"wg{i}")
                for k in range(kch):
                    nc.sync.dma_start(t[:, k, :], WG[i][k * 128:(k + 1) * 128, :])
                wg_sb.append(t)
            bg_sb = [load_const(bG[i], (128, Ls[i]["HC"] if i == 0 else Ls[i]["C"]),
                                F32, f"bg{i}") for i in range(3)]
            wm1 = cpool.tile([128, 2, cfg.HID], BF16, tag="wm1")
            for k in range(2):
                nc.sync.dma_start(wm1[:, k, :], Wm1[k * 128:(k + 1) * 128, :])
            wm2 = load_const(Wm2, (cfg.HID, cfg.HID), BF16, "wm2")
            b1s = load_const(b1c, (cfg.HID, 1), F32, "b1s")
            b2s = load_const(b2c, (cfg.HID, 1), F32, "b2s")

            tbl_inA = [dpool.tile([cfg.nA, Ls[i]["row"]], BF16, tag=f"tinA{i}",
                                  name=f"tinA{i}")
                       for i in range(3)]
            tbl_inB = [dpool.tile([cfg.nB, Ls[i]["row"]], BF16, tag=f"tinB{i}",
                                  name=f"tinB{i}")
                       for i in range(3)]
            tbl_agA = [dpool.tile([cfg.nA * cfg.n_cores, Ls[i]["row"]], BF16,
                                  tag=f"tagA{i}", name=f"tagA{i}",
                                  addr_space="Shared")
                       for i in range(3)]
            tbl_agB = [dpool.tile([cfg.nB * cfg.n_cores, Ls[i]["row"]], BF16,
                                  tag=f"tagB{i}", name=f"tagB{i}",
                                  addr_space="Shared")
                       for i in range(3)]

            ad_tiles = {}

            def node_chunk(li, j, lh, koff=0):
                """Project SBUF-resident h columns (lh[:, koff+k, :nn]) of
                window j into table rows and store to local DRAM table."""
                L = Ls[li]
                kch = L["in_ch"] // 128
                NCOL = L["HC"] + 2 * L["H"]
                ROW = L["row"]
                n0 = j * 128
                nn = cfg.win_size(j)
                ps = psC.tile([128, NCOL], F32, tag="mm", name="psn")
                for k in range(kch):
                    nc.tensor.matmul(ps[:nn, :], lh[:, koff + k, :nn],
                                     wg_sb[li][:, k, :],
                                     start=(k == 0), stop=(k == kch - 1))
                tb = npool.tile([128, ROW], BF16, tag="tb", name="tb")
                nc.vector.tensor_copy(tb[:nn, :NCOL], ps[:nn, :])
                if j < cfg.WA:
                    nc.sync.dma_start(
                        tbl_inA[li][n0:n0 + nn, :NCOL], tb[:nn, :NCOL])
                else:
                    m0 = n0 - cfg.nA
                    nc.sync.dma_start(
                        tbl_inB[li][m0:m0 + nn, :NCOL], tb[:nn, :NCOL])

            def emit_ag(li, half):
                if half == 0:
                    nc.gpsimd.collective_compute(
                        "AllGather", ALU.bypass, replica_groups=rep,
                        ins=[tbl_inA[li].opt()], outs=[tbl_agA[li].opt()])
                else:
                    nc.gpsimd.collective_compute(
                        "AllGather", ALU.bypass, replica_groups=rep,
                        ins=[tbl_inB[li].opt()], outs=[tbl_agB[li].opt()])

            def emit_ad(li):
                L = Ls[li]
                H = L["H"]
                ad_all = spool.tile([128, W, MAXH], BF16, tag="ad_all",
                                    name="ad_all", bufs=2)
                ad_tiles[li] = ad_all
                nc.vector.memset(ad_all[:], 0.0)
                ad_fA = tbl_inA[li][:, L["ad_off"]:L["ad_off"] + H]
                nc.sync.dma_start(
                    ad_all[:, :cfg.WA, :H],
                    ad_fA.rearrange("(w q) h -> q w h", q=128))
                full_b = cfg.nB // 128
                if full_b:
                    ad_fB = tbl_inB[li][:full_b * 128,
                                        L["ad_off"]:L["ad_off"] + H]
                    nc.sync.dma_start(
                        ad_all[:, cfg.WA:cfg.WA + full_b, :H],
                        ad_fB.rearrange("(w q) h -> q w h", q=128))
                if cfg.nB % 128:
                    rem = cfg.nB - full_b * 128
                    nc.sync.dma_start(
                        ad_all[:rem, cfg.WA + full_b, :H],
                        tbl_inB[li][full_b * 128:,
                                    L["ad_off"]:L["ad_off"] + H])

            # ===== MLP (node-sharded), fused with layer-0 node phase =====
            CH = 512
            nch = math.ceil(n_loc / CH)
            for j in range(nch):
                n0 = j * CH
                nn = min(CH, n_loc - n0)
                xt = npool.tile([128, 2, CH], BF16, tag="xt")
                for k in range(2):
                    nc.sync.dma_start(xt[:, k, :nn],
                                      xT[k * 128:(k + 1) * 128, n0:n0 + nn])
                ps = psC.tile([128, CH], F32, tag="mm")
                for k in range(2):
                    nc.tensor.matmul(ps[:, :nn], wm1[:, k, :], xt[:, k, :nn],
                                     start=(k == 0), stop=(k == 1))
                h1 = npool.tile([128, CH], BF16, tag="h1")
                nc.scalar.activation(h1[:, :nn], ps[:, :nn], AF.Relu,
                                     bias=b1s[:, 0:1])
                ps2 = psC.tile([128, CH], F32, tag="mm")
                nc.tensor.matmul(ps2[:, :nn], wm2[:, :], h1[:, :nn],
                                 start=True, stop=True)
                h2 = npool.tile([128, 4, 128], BF16, tag="h2")
                nc.scalar.activation(h2.rearrange("p a b -> p (a b)")[:, :nn],
                                     ps2[:, :nn], AF.Relu, bias=b2s[:, 0:1])
                for w in range(4 * j, min(4 * j + 4, W)):
                    node_chunk(0, w, h2, koff=w - 4 * j)
                    if w == cfg.WA - 1:
                        emit_ag(0, 0)

            emit_ag(0, 1)
            emit_ad(0)

            # ===== layers: flat window stream with lo-gather lookahead =====
            LOOK = getattr(cfg, "look", 10)
            GMAX = 8  # tiles per dma_gather (>1024 idxs crashes HW)
            TLOMAX = max(p.T_lo)
            THIMAX = max(p.T_hi)
            GB = max(L["row"] for L in Ls)
            seq = [(li, t) for li in range(3) for t in range(W)]
            glo_tiles = {}
            ghi_tiles = {}
            qrr = [0]  # SWDGE queue round-robin state

            def next_q():
                q = qrr[0]
                qrr[0] = (q + 1) % cfg.NQ
                return q

            # one-time memset of gather pools: with -1-truncated idx padding,
            # unwritten slots must hold bounded values (exp() stays finite)
            for _ in range(getattr(cfg, "lo_bufs", 7)):
                z = glopool.tile([128, TLOMAX * GB], BF16, tag="glo",
                                 name="glo")
                nc.vector.memset(z[:], 0.0)
            for _ in range(getattr(cfg, "hi_bufs", 3)):
                z = ghipool.tile([128, (THIMAX + 1) * GB], BF16, tag="ghi",
                                 name="ghi")
                nc.vector.memset(z[:], 0.0)

            def load_cnt(t, half, q0, q):
                """Load this call's per-core actual idx count into cnt_reg
                (same-engine FIFO orders it before the gather decode)."""
                slot = len(p.call_order)
                p.call_order.append((t, half, q0, q))
                nc.gpsimd.reg_load(cnt_reg, gcnt[0:1, slot:slot + 1])
                return cnt_reg

            def emit_lo(li, t):
                ROW = Ls[li]["row"]
                Tlo = p.T_lo[t]
                raw = glopool.tile([128, TLOMAX * GB], BF16, tag="glo",
                                   name="glo")
                glo = raw[:, :Tlo * ROW].rearrange("p (a b) -> p a b", b=ROW)
                glo_tiles[(li, t)] = glo
                for q0 in range(0, Tlo, GMAX):
                    q = min(GMAX, Tlo - q0)
                    r = load_cnt(t, 0, q0, q)
                    nc.gpsimd.dma_gather(
                        glo[:, q0:q0 + q, :], tbl_agA[li][:, :],
                        idx_lo[:, p.win_lo_off[t] + q0 * 8:
                               p.win_lo_off[t] + (q0 + q) * 8],
                        q * 128, r, ROW, queue_num=next_q())

            def emit_hi(li, t):
                L = Ls[li]
                ROW = L["row"]
                NCOL = L["HC"] + 2 * L["H"]
                Thi = p.T_hi[t]
                nn = cfg.win_size(t)
                raw = ghipool.tile([128, (THIMAX + 1) * GB], BF16, tag="ghi",
                                   name="ghi")
                ghi = raw[:, :(Thi + 1) * ROW].rearrange("p (a b) -> p a b",
                                                         b=ROW)
                ghi_tiles[(li, t)] = ghi
                # self-loop rows: direct sequential load, no gather
                if t < cfg.WA:
                    nc.sync.dma_start(
                        ghi[:nn, Thi, :NCOL],
                        tbl_inA[li][t * 128:t * 128 + nn, :NCOL])
                else:
                    m0 = t * 128 - cfg.nA
                    nc.sync.dma_start(
                        ghi[:nn, Thi, :NCOL],
                        tbl_inB[li][m0:m0 + nn, :NCOL])
                for q0 in range(0, Thi, GMAX):
                    q = min(GMAX, Thi - q0)
                    r = load_cnt(t, 1, q0, q)
                    nc.gpsimd.dma_gather(
                        ghi[:, q0:q0 + q, :], tbl_agB[li][:, :],
                        idx_hi[:, p.win_hi_off[t] + q0 * 8:
                               p.win_hi_off[t] + (q0 + q) * 8],
                        q * 128, r, ROW, queue_num=next_q())

            def compute(li, t):
                L = Ls[li]
                H, Cc, HC, ROW = L["H"], L["C"], L["HC"], L["row"]
                NCOL = HC + 2 * H
                ad_all = ad_tiles[li]
                Tlo, Thi = p.T_lo[t], p.T_hi[t]
                T = Tlo + Thi + 1
                nn = cfg.win_size(t)
                to = p.win_tile_off[t]
                glo = glo_tiles.pop((li, t))
                ghi = ghi_tiles.pop((li, t))

                oT = opool.tile([128, T, 128], BF16, tag="oT", name="oT")
                oD = opool.tile([128, T, 128], BF16, tag="oD", name="oD")
                for k0 in range(0, T, cfg.K_FUSE):
                    K = min(cfg.K_FUSE, T - k0)
                    nc.vector.tensor_tensor(
                        oT[:, k0:k0 + K, :],
                        iota_row.unsqueeze(1).broadcast_to([128, K, 128]),
                        dstcol[:, to + k0:to + k0 + K]
                            .unsqueeze(2).broadcast_to([128, K, 128]),
                        ALU.is_equal)
                    ptd = psC.tile([128, cfg.K_FUSE, 128], BF16, tag="ptd",
                                   name="ptd", bufs=2)
                    for i in range(K):
                        nc.tensor.transpose(ptd[:, i, :],
                                            oT[:, k0 + i, :], ident[:, :])
                    nc.scalar.activation(oD[:, k0:k0 + K, :],
                                         ptd[:, :K, :], AF.Copy)

                ps_ad = psB.tile([128, MAXT * MAXH], F32, tag="ps_ad")
                for i in range(T):
                    nc.tensor.matmul(ps_ad[:, i * H:(i + 1) * H],
                                     oD[:, i, :], ad_all[:, t, :H],
                                     start=True, stop=True)
                e_sb = spool.tile([128, MAXT * MAXH], F32, tag="e_sb")
                nc.vector.tensor_tensor(
                    e_sb[:, :Tlo * H], ps_ad[:, :Tlo * H],
                    glo[:, 0:Tlo, L["as_off"]:L["as_off"] + H],
                    ALU.add)
                nc.vector.tensor_tensor(
                    e_sb[:, Tlo * H:T * H], ps_ad[:, Tlo * H:T * H],
                    ghi[:, 0:Thi + 1, L["as_off"]:L["as_off"] + H],
                    ALU.add)
                ex1 = spool.tile([128, MAXT * MAXH], F32, tag="ex1")
                nc.scalar.activation(ex1[:, :T * H], e_sb[:, :T * H], AF.Exp)
                ex2 = spool.tile([128, MAXT * MAXH], F32, tag="ex2")
                nc.scalar.activation(ex2[:, :T * H], e_sb[:, :T * H], AF.Exp,
                                     scale=NEG_SLOPE)
                msg = mpool.tile([128, T, HC + H], BF16, tag="msg")
                nc.vector.tensor_tensor(
                    msg[:, 0:T, HC:HC + H],
                    ex1[:, :T * H], ex2[:, :T * H], ALU.max)
                # h' is (c,h)-interleaved: the per-head weight broadcasts on
                # the MIDDLE dim (c) -> DVE 2x mode stays active
                for k0 in range(0, Tlo, cfg.K_FUSE):
                    K = min(cfg.K_FUSE, Tlo - k0)
                    nc.vector.tensor_tensor(
                        msg[:, k0:k0 + K, 0:HC],
                        glo[:, k0:k0 + K, 0:HC],
                        msg[:, k0:k0 + K, HC:HC + H]
                            .unsqueeze(2).broadcast_to([128, K, Cc, H]),
                        ALU.mult)
                for k0 in range(0, Thi + 1, cfg.K_FUSE):
                    K = min(cfg.K_FUSE, Thi + 1 - k0)
                    nc.vector.tensor_tensor(
                        msg[:, Tlo + k0:Tlo + k0 + K, 0:HC],
                        ghi[:, k0:k0 + K, 0:HC],
                        msg[:, Tlo + k0:Tlo + k0 + K, HC:HC + H]
                            .unsqueeze(2).broadcast_to([128, K, Cc, H]),
                        ALU.mult)
                ps_w = psA.tile([128, HC + H], F32, tag="ps_w")
                for i in range(T):
                    nc.tensor.matmul(ps_w[:, :], oT[:, i, :],
                                     msg[:, i, :],
                                     start=(i == 0), stop=(i == T - 1))
                rcp = spool.tile([128, MAXH], F32, tag="rcp")
                nc.vector.reciprocal(rcp[:, :H], ps_w[:, HC:HC + H])
                if li == 1:
                    nc.vector.tensor_scalar_mul(rcp[:, :H], rcp[:, :H], 0.5)
                y = spool.tile([128, MAXHC], F32, tag="y")
                nc.vector.tensor_tensor(
                    y[:, :HC], ps_w[:, :HC],
                    rcp[:, :H].unsqueeze(1).broadcast_to([128, Cc, H]),
                    ALU.mult)
                if li == 1:
                    # head mean over the interleaved (c,h) pairs
                    yv = y[:, :HC].rearrange("p (c h) -> p c h", h=H)
                    ym = spool.tile([128, MAXHC], F32, tag="ym")
                    nc.vector.tensor_tensor(ym[:, :Cc], yv[:, :, 0],
                                            yv[:, :, 1], ALU.add)
                    ycols = Cc
                    ysrc = ym
                else:
                    ycols = HC
                    ysrc = y
                nc.vector.tensor_tensor(
                    y[:, :ycols], ysrc[:, :ycols],
                    bg_sb[li][:, :ycols], ALU.add)
                if li < 2:
                    e1 = spool.tile([128, MAXHC], F32, tag="elu1")
                    nc.scalar.activation(e1[:, :ycols], y[:, :ycols], AF.Exp)
                    nc.scalar.activation(e1[:, :ycols], e1[:, :ycols],
                                         AF.Relu, scale=-1.0, bias=1.0)
                    nc.scalar.activation(y[:, :ycols], y[:, :ycols], AF.Relu)
                    yb = spool.tile([128, MAXHC], BF16, tag="yb")
                    nc.vector.tensor_tensor(yb[:, :ycols], y[:, :ycols],
                                            e1[:, :ycols], ALU.subtract)
                    pts = npool.tile([128, ycols // 128, 128], BF16,
                                     tag="pts", name="pts")
                    for k in range(ycols // 128):
                        pt = psC.tile([128, 128], BF16, tag="mm")
                        nc.tensor.transpose(pt[:, :],
                                            yb[:, k * 128:(k + 1) * 128],
                                            ident[:, :])
                        nc.vector.tensor_copy(pts[:, k, :], pt[:, :])
                    node_chunk(li + 1, t, pts)
                    if t == cfg.WA - 1:
                        emit_ag(li + 1, 0)
                    if t == W - 1:
                        emit_ag(li + 1, 1)
                        emit_ad(li + 1)
                else:
                    nc.sync.dma_start(out_d[t * 128:t * 128 + nn, :],
                                      y[:nn, :ycols])

            for s0 in range(min(LOOK, len(seq))):
                emit_lo(*seq[s0])
            for step, (li, t) in enumerate(seq):
                if step + LOOK < len(seq):
                    emit_lo(*seq[step + LOOK])
                emit_hi(li, t)
                compute(li, t)
    return nc, out_d


# ---------------------------------------------------------------- runner

def make_inmaps(inputs, cfg: Cfg, p: Plan):
    wts = prep_weights(inputs, cfg)
    x = np.asarray(inputs["x"], np.float32)
    xT = np.ascontiguousarray(x.T).astype(bf)
    n_loc = cfg.n_loc
    in_maps = []
    for c in range(cfg.n_cores):
        m = dict(wts)
        m["xT"] = np.ascontiguousarray(
            xT[:, c * n_loc:(c + 1) * n_loc][:, p.perm[c]])
        m["idx_lo"] = p.idx_lo[c]
        m["idx_hi"] = p.idx_hi[c]
        import os as _os
        if _os.environ.get("TRUNC_OFF"):
            m["gcnt"] = np.array(
                [[q * 128 for (t, half, q0, q) in p.call_order]], np.uint32)
        else:
            m["gcnt"] = np.array(
                [[min(max(p.blen[c][(t, half)] - q0 * 128, 0), q * 128)
                  for (t, half, q0, q) in p.call_order]], np.uint32)
        m["dstcol"] = p.dstloc_col[c]
        in_maps.append(m)
    return in_maps


def build_program(cfg: Cfg, p: Plan, debug=False):
    nc = bacc.Bacc("TRN2", target_bir_lowering=False, debug=debug,
                   num_devices=cfg.n_cores, num_swdge_queues=cfg.NQ)
    build(nc, cfg, p)
    nc.compile()
    return nc


# ------------------------------------------------------------- entry point

_CACHE = {}


def kernel(**inputs):
    import numpy as _np
    from concourse.bass_utils import run_bass_kernel_spmd

    cfg = Cfg()
    ei = _np.asarray(inputs["edge_index"])
    key = hash(ei.tobytes())
    if key not in _CACHE:
        p = plan_edges(ei, cfg)
        nc = build_program(cfg, p, debug=False)
        _CACHE[key] = (p, nc)
    p, nc = _CACHE[key]
    in_maps = make_inmaps(inputs, cfg, p)
    res = run_bass_kernel_spmd(nc, in_maps, list(range(cfg.n_cores)))
    n_loc = cfg.n_loc
    out = _np.empty((cfg.N, cfg.OUT), _np.float32)
    for c in range(cfg.n_cores):
        out[c * n_loc + _np.asarray(p.perm[c])] = \
            res.results[c]["out"].astype(_np.float32)
    return out



# revision 51
# speedup vs baseline: 1.1524x; 1.1524x over previous
"""Trainium2 Bass kernel: 3-layer GAT message passing, 8-core SPMD.

Dst-sharded edge phase (dma_gather + one-hot scatter matmuls into PSUM;
self-loop rows bulk-loaded, bypassing gather descriptors), node-sharded
table build interleaved into the previous edge phase, split AllGathers.
"""
"""GAT message-passing kernel for TRN2, 8-core SPMD.

Per layer:
  node phase (node-sharded): table rows [h' | alpha_s | alpha_d | pad] for own
  nodes via PE matmuls with host-packed rhs [W | U_s | U_d].
  AllGather table -> full gather table in every core's DRAM.
  edge phase (dst-sharded): per dst-window of 128 nodes, dma_gather rows for
  the window's edges (src-indexed, int16 lo/hi split), build one-hot
  scatter/expand matrices on DVE, expand alpha_d via PE matmul,
  w = max(exp(e), exp(slope*e)) == exp(leaky_relu(e)), weighted messages
  scatter-accumulated into PSUM via one-hot matmul, normalized by the
  softmax denominator (segment max subtraction dropped; shift-invariant).
"""

import math
from dataclasses import dataclass, field

import numpy as np
import ml_dtypes

import concourse.bass as bass
import concourse.bacc as bacc
import concourse.mybir as mybir
import concourse.tile as tile
from concourse import library_config

F32 = mybir.dt.float32
BF16 = mybir.dt.bfloat16
I16 = mybir.dt.int16
AF = mybir.ActivationFunctionType
ALU = mybir.AluOpType
NEG_SLOPE = 0.2

bf = ml_dtypes.bfloat16


@dataclass
class Cfg:
    N: int = 50000
    n_cores: int = 8
    IN: int = 256
    HID: int = 128
    OUT: int = 128
    TSPLIT: int = 32768
    K_FUSE: int = 8
    NQ: int = 4  # SWDGE queues: round-robin descgen across Q7 core pairs

    @property
    def n_loc(self):
        return self.N // self.n_cores

    @property
    def n_win(self):
        return (self.n_loc + 127) // 128

    def win_size(self, t):
        return min(128, self.n_loc - t * 128)

    @property
    def WA(self):
        return 26

    @property
    def nA(self):
        return min(self.WA * 128, self.n_loc)

    @property
    def nB(self):
        return self.n_loc - self.nA

    # table pieces: each table's AllGather is split so the LAST piece is
    # small -- its wire time is the only part not hidden behind the
    # producer's remaining window projections (gathers wait on ALL pieces,
    # but earlier pieces finish during later writes).
    @property
    def WA1(self):
        return 13          # windows in piece A1 (A2 = WA - WA1)

    @property
    def WB1(self):
        return 17          # windows in piece B1 (B2 = rest)

    @property
    def nA1(self):
        return self.WA1 * 128

    @property
    def nA2(self):
        return self.nA - self.nA1

    @property
    def nB1(self):
        return self.WB1 * 128

    @property
    def nB2(self):
        return self.nB - self.nB1

    @property
    def layers(self):
        hid, out = self.HID, self.OUT
        ls = []
        for (H, C, in_ch) in ((4, hid // 2, hid), (2, out, 2 * hid), (1, out, out)):
            HC = H * C
            row = HC + 2 * H
            row_pad = ((row * 2 + 255) // 256) * 256 // 2
            ls.append(dict(H=H, C=C, in_ch=in_ch, HC=HC, row=row_pad,
                           as_off=HC, ad_off=HC + H))
        return ls


# ---------------------------------------------------------------- host plan

@dataclass
class Plan:
    T_lo: list
    T_hi: list
    idx_lo: list
    idx_hi: list
    dstloc_col: list
    dstloc_row: list
    NT: int = 0
    cols_lo: int = 0
    cols_hi: int = 0
    win_tile_off: list = field(default_factory=list)
    win_lo_off: list = field(default_factory=list)
    win_hi_off: list = field(default_factory=list)


def plan_edges(edge_index, cfg: Cfg) -> Plan:
    """Balanced plan: per core, pack local dst nodes into windows so that
    per-window (lo, hi) edge sums are near-uniform across windows AND cores
    (kills ceil + max-over-core tile padding).  Nodes with original local
    index < nA go to windows [0, WA) (table A), the rest to [WA, W) (B) --
    membership is decided by original index so lo/hi degrees are fixed
    before packing (no circularity).  perm[c][p] = original local node at
    permuted position p; inputs (xT) are host-permuted, outputs unpermuted
    in kernel()."""
    N, C = cfg.N, cfg.n_cores
    n_loc = cfg.n_loc
    W = cfg.n_win
    src = np.asarray(edge_index[0], np.int64)      # self loops handled by a
    dst = np.asarray(edge_index[1], np.int64)      # direct per-window load
    s_core = src // n_loc
    s_loc = src % n_loc
    lo_edge = s_loc < cfg.nA                       # src in table A?
    d_core = dst // n_loc
    d_loc = dst % n_loc

    # per-node lo/hi in-degree
    lo_deg = np.zeros(N, np.int64)
    hi_deg = np.zeros(N, np.int64)
    np.add.at(lo_deg, dst[lo_edge], 1)
    np.add.at(hi_deg, dst[~lo_edge], 1)

    # per-core packing
    perm = []        # perm[c][p] = original local idx at permuted pos p
    win_of = np.zeros(N, np.int64)
    slot_of = np.zeros(N, np.int64)
    for c in range(C):
        ld = lo_deg[c * n_loc:(c + 1) * n_loc]
        hd = hi_deg[c * n_loc:(c + 1) * n_loc]
        pc = np.zeros(n_loc, np.int64)
        import os
        identity = bool(int(os.environ.get("PLAN_IDENTITY", "0")))
        for half, (wins, nodes) in enumerate((
                (np.arange(cfg.WA), np.arange(cfg.nA)),
                (np.arange(cfg.WA, W), np.arange(cfg.nA, n_loc)))):
            caps = np.array([cfg.win_size(w) for w in wins], np.int64)
            lt = np.zeros(len(wins))
            ht = np.zeros(len(wins))
            space = caps.copy()
            # per-window targets proportional to window size
            lo_t = ld[nodes].sum() * caps / caps.sum()
            hi_t = hd[nodes].sum() * caps / caps.sum()
            order = (nodes if identity else
                     nodes[np.argsort(-(ld[nodes] + hd[nodes]), kind="stable")])
            fill_ptr = 0
            for nloc_i in order:
                l, h = ld[nloc_i], hd[nloc_i]
                if identity:
                    if space[fill_ptr] <= 0:
                        fill_ptr += 1
                    wi = fill_ptr
                else:
                    score = np.maximum((lt + l) - lo_t, (ht + h) - hi_t)
                    score[space <= 0] = 1 << 30
                    wi = int(np.argmin(score))
                lt[wi] += l
                ht[wi] += h
                space[wi] -= 1
                w = wins[wi]
                slot = cfg.win_size(w) - space[wi] - 1
                win_of[c * n_loc + nloc_i] = w
                slot_of[c * n_loc + nloc_i] = slot
                pc[(w * 128 if w < cfg.WA else
                    cfg.nA + (w - cfg.WA) * 128) + slot] = nloc_i
        perm.append(pc)

    # permuted position of each node within its owner's table
    pos = np.zeros(N, np.int64)
    for c in range(C):
        pc = perm[c]
        pos[c * n_loc + pc] = np.arange(n_loc)
    idx_a = s_core * cfg.nA + pos[src]             # valid when lo_edge
    idx_b = s_core * cfg.nB + (pos[src] - cfg.nA)  # valid when ~lo_edge

    win = win_of[dst]
    dloc = slot_of[dst]
    core = d_core

    buckets = {}
    for c in range(C):
        m_c = core == c
        for t in range(W):
            m = m_c & (win == t)
            ml = m & lo_edge
            mh = m & ~lo_edge
            sl, dll = idx_a[ml], dloc[ml]
            o = np.argsort(sl, kind="stable")
            buckets[(c, t, 0)] = (sl[o], dll[o])
            sh, dlh = idx_b[mh], dloc[mh]
            o = np.argsort(sh, kind="stable")
            buckets[(c, t, 1)] = (sh[o], dlh[o])

    T_lo = [max(math.ceil(len(buckets[(c, t, 0)][0]) / 128) for c in range(C))
            for t in range(W)]
    T_hi = [max(math.ceil(len(buckets[(c, t, 1)][0]) / 128) for c in range(C))
            for t in range(W)]

    NT = sum(T_lo) + sum(T_hi) + W  # +1 self tile per window
    p = Plan(T_lo=T_lo, T_hi=T_hi, idx_lo=[], idx_hi=[], dstloc_col=[],
             dstloc_row=[], NT=NT, cols_lo=max(sum(T_lo) * 8, 8),
             cols_hi=max(sum(T_hi) * 8, 8))
    p.perm = perm
    off = olo = ohi = 0
    for t in range(W):
        p.win_tile_off.append(off)
        p.win_lo_off.append(olo)
        p.win_hi_off.append(ohi)
        off += T_lo[t] + T_hi[t] + 1
        olo += T_lo[t] * 8
        ohi += T_hi[t] * 8

    for c in range(C):
        ilo = np.zeros((128, p.cols_lo), np.int16)
        ihi = np.zeros((128, p.cols_hi), np.int16)
        dcol = np.full((128, NT), -1.0, np.float32)
        for t in range(W):
            for half, (idx_arr, col_off, Tn) in enumerate(
                ((ilo, p.win_lo_off[t], T_lo[t]),
                 (ihi, p.win_hi_off[t], T_hi[t]))):
                if Tn == 0:
                    continue
                s_, dl = buckets[(c, t, half)]
                n = Tn * 128
                # pad with 0 (gathers row 0): -1 runtime truncation desyncs
                # the decode-side SWDGE ring booking from the Q7's writes
                si = np.zeros(n, np.int64)
                si[:len(s_)] = s_
                dli = np.full(n, -1.0, np.float32)
                dli[:len(dl)] = dl
                blk = si.astype(np.int16).reshape(Tn * 8, 16).T
                idx_arr[:, col_off:col_off + Tn * 8] = np.tile(blk, (8, 1))
                tb = p.win_tile_off[t] + (0 if half == 0 else T_lo[t])
                dcol[:, tb:tb + Tn] = dli.reshape(Tn, 128).T
            ts = p.win_tile_off[t] + T_lo[t] + T_hi[t]
            nn_w = min(128, n_loc - t * 128)
            selfcol = np.full(128, -1.0, np.float32)
            selfcol[:nn_w] = np.arange(nn_w)
            dcol[:, ts] = selfcol
        p.idx_lo.append(ilo)
        p.idx_hi.append(ihi)
        p.dstloc_col.append(dcol.astype(bf))
        p.dstloc_row.append(dcol.T.reshape(1, NT * 128).astype(bf))
    return p


# ------------------------------------------------------------- host weights

def prep_weights(inp, cfg: Cfg):
    """Pack per-layer projection weights.  h' columns are stored
    (c, h)-interleaved (head innermost) so the per-head message weighting
    broadcasts on a MIDDLE dim -- DVE runs 2x faster than with an
    innermost-dim broadcast."""
    out = {}
    for li, (wk, ak, dk) in enumerate(
            (("g1_W", "g1_as", "g1_ad"), ("g2_W", "g2_as", "g2_ad"),
             ("g3_W", "g3_as", "g3_ad"))):
        L = cfg.layers[li]
        Wm = np.asarray(inp[wk], np.float32)
        if li == 1:
            # layer-1 input = layer-0 output, which we emit (c,h)-interleaved
            L0 = cfg.layers[0]
            j = np.arange(L0["HC"])
            perm_in = (j % L0["H"]) * L0["C"] + j // L0["H"]
            Wm = Wm[perm_in]
        a_s = np.asarray(inp[ak], np.float32)
        a_d = np.asarray(inp[dk], np.float32)
        H, Cc = L["H"], L["C"]
        U_s = np.stack([Wm[:, h * Cc:(h + 1) * Cc] @ a_s[h] for h in range(H)], 1)
        U_d = np.stack([Wm[:, h * Cc:(h + 1) * Cc] @ a_d[h] for h in range(H)], 1)
        # (h,c) -> (c,h) column interleave
        Wm_i = np.ascontiguousarray(
            Wm.reshape(-1, H, Cc).transpose(0, 2, 1).reshape(Wm.shape))
        out[f"WG{li+1}"] = np.concatenate([Wm_i, U_s, U_d], 1).astype(bf)
        bias = np.asarray(inp[f"g{li+1}_b"], np.float32)
        if li == 0:
            bias = np.ascontiguousarray(
                bias.reshape(H, Cc).T.reshape(-1))
        out[f"bG{li+1}"] = np.ascontiguousarray(np.broadcast_to(
            bias[None, :], (128, len(bias))))
    out["Wm1"] = np.asarray(inp["W1"], np.float32).astype(bf)
    out["Wm2"] = np.asarray(inp["W2"], np.float32).astype(bf)
    out["b1c"] = np.ascontiguousarray(np.asarray(inp["b1"], np.float32)[:, None])
    out["b2c"] = np.ascontiguousarray(np.asarray(inp["b2"], np.float32)[:, None])
    out["iota_row"] = np.ascontiguousarray(
        np.broadcast_to(np.arange(128, dtype=np.float32), (128, 128))).astype(bf)
    out["iota_col"] = np.arange(128, dtype=np.float32)[:, None].astype(bf)
    out["ident"] = np.eye(128, dtype=np.float32).astype(bf)
    return out


# ---------------------------------------------------------------- builder

def build(nc, cfg: Cfg, p: Plan, dbg=False):
    W = cfg.n_win
    n_loc = cfg.n_loc
    Ls = cfg.layers
    MAXT = max(p.T_lo[t] + p.T_hi[t] for t in range(W)) + 1
    MAXHC = max(L["HC"] for L in Ls)
    MAXH = max(L["H"] for L in Ls)

    def din(name, shape, dt):
        return nc.dram_tensor(name, list(shape), dt, kind="ExternalInput")

    xT = din("xT", (cfg.IN, n_loc), BF16)
    Wm1 = din("Wm1", (cfg.IN, cfg.HID), BF16)
    Wm2 = din("Wm2", (cfg.HID, cfg.HID), BF16)
    b1c = din("b1c", (cfg.HID, 1), F32)
    b2c = din("b2c", (cfg.HID, 1), F32)
    WG = [din(f"WG{i+1}", (Ls[i]["in_ch"], Ls[i]["HC"] + 2 * Ls[i]["H"]), BF16)
          for i in range(3)]
    bG = [din(f"bG{i+1}", (128, Ls[i]["HC"] if i == 0 else Ls[i]["C"]), F32)
          for i in range(3)]
    iota_row_d = din("iota_row", (128, 128), BF16)
    iota_col_d = din("iota_col", (128, 1), BF16)
    ident_d = din("ident", (128, 128), BF16)
    idx_lo_d = din("idx_lo", (128, p.cols_lo), I16)
    idx_hi_d = din("idx_hi", (128, p.cols_hi), I16)
    dstcol_d = din("dstcol", (128, p.NT), BF16)
    out_d = nc.dram_tensor("out", [n_loc, cfg.OUT], F32, kind="ExternalOutput")
    dbg_d = {}
    if dbg:
        L0 = Ls[0]
        T0 = p.T_lo[0] + p.T_hi[0]
        for nm, shape in (("d_piece0", (cfg.HID, n_loc)),
                          ("d_tin0", (n_loc, L0["row"])),
                          ("d_tag0", (cfg.N, L0["row"])),
                          ("d_g0", (128, T0 * L0["row"])),
                          ("d_oT0", (128, T0 * 128)),
                          ("d_oD0", (128, T0 * 128)),
                          ("d_esb0", (128, T0 * L0["H"])),
                          ("d_msg0", (128, T0 * (L0["HC"] + L0["H"]))),
                          ("d_psw0", (128, L0["HC"] + L0["H"])),
                          ("d_ad0", (128, L0["H"]))):
            dt = BF16 if nm in ("d_tin0", "d_tag0", "d_g0", "d_oT0", "d_oD0",
                                "d_msg0", "d_piece0", "d_ad0") else F32
            dbg_d[nm] = nc.dram_tensor(nm, list(shape), dt, kind="ExternalOutput")

    rep = [list(range(cfg.n_cores))]

    with tile.TileContext(nc) as tc:
        with (
            tc.tile_pool(name="const", bufs=1) as cpool,
            tc.tile_pool(name="glo", bufs=getattr(cfg, "lo_bufs", 12)) as glopool,
            tc.tile_pool(name="ghi", bufs=getattr(cfg, "hi_bufs", 3)) as ghipool,
            tc.tile_pool(name="oh", bufs=2) as opool,
            tc.tile_pool(name="msg", bufs=2) as mpool,
            tc.tile_pool(name="small", bufs=4) as spool,
            tc.tile_pool(name="nodes", bufs=3) as npool,
            tc.tile_pool(name="psA", bufs=getattr(cfg, "psA_bufs", 2), space="PSUM") as psA,
            tc.tile_pool(name="psB", bufs=getattr(cfg, "psB_bufs", 2), space="PSUM") as psB,
            tc.tile_pool(name="psC", bufs=2, space="PSUM") as psC,
            tc.tile_pool(name="dram", bufs=1, space="DRAM") as dpool,
        ):
            nc.gpsimd.load_library(library_config.mlp)

            def load_const(handle, shape, dtp, tag):
                t = cpool.tile(list(shape), dtp, tag=tag, name=tag)
                nc.sync.dma_start(t[:], handle[:])
                return t

            iota_row = load_const(iota_row_d, (128, 128), BF16, "iota_row")
            iota_col = load_const(iota_col_d, (128, 1), BF16, "iota_col")
            ident = load_const(ident_d, (128, 128), BF16, "ident")
            idx_lo = load_const(idx_lo_d, (128, p.cols_lo), I16, "idx_lo")
            idx_hi = load_const(idx_hi_d, (128, p.cols_hi), I16, "idx_hi")
            dstcol = load_const(dstcol_d, (128, p.NT), BF16, "dstcol")
            wg_sb = []
            for i in range(3):
                L = Ls[i]
                kch = L["in_ch"] // 128
                t = cpool.tile([128, kch, L["HC"] + 2 * L["H"]], BF16, tag=f"wg{i}",
                               name=f"wg{i}")
                for k in range(kch):
                    nc.sync.dma_start(t[:, k, :], WG[i][k * 128:(k + 1) * 128, :])
                wg_sb.append(t)
            bg_sb = [load_const(bG[i], (128, Ls[i]["HC"] if i == 0 else Ls[i]["C"]),
                                F32, f"bg{i}") for i in range(3)]
            wm1 = cpool.tile([128, 2, cfg.HID], BF16, tag="wm1")
            for k in range(2):
                nc.sync.dma_start(wm1[:, k, :], Wm1[k * 128:(k + 1) * 128, :])
            wm2 = load_const(Wm2, (cfg.HID, cfg.HID), BF16, "wm2")
            b1s = load_const(b1c, (cfg.HID, 1), F32, "b1s")
            b2s = load_const(b2c, (cfg.HID, 1), F32, "b2s")

            tbl_inA = [dpool.tile([cfg.nA, Ls[i]["row"]], BF16, tag=f"tinA{i}",
                                  name=f"tinA{i}")
                       for i in range(3)]
            tbl_inB = [dpool.tile([cfg.nB, Ls[i]["row"]], BF16, tag=f"tinB{i}",
                                  name=f"tinB{i}")
                       for i in range(3)]
            tbl_agA = [dpool.tile([cfg.nA * cfg.n_cores, Ls[i]["row"]], BF16,
                                  tag=f"tagA{i}", name=f"tagA{i}",
                                  addr_space="Shared")
                       for i in range(3)]
            tbl_agB = [dpool.tile([cfg.nB * cfg.n_cores, Ls[i]["row"]], BF16,
                                  tag=f"tagB{i}", name=f"tagB{i}",
                                  addr_space="Shared")
                       for i in range(3)]

            ad_tiles = {}

            def node_chunk(li, j, lh, koff=0):
                """Project SBUF-resident h columns (lh[:, koff+k, :nn]) of
                window j into table rows and store to local DRAM table."""
                L = Ls[li]
                kch = L["in_ch"] // 128
                NCOL = L["HC"] + 2 * L["H"]
                ROW = L["row"]
                n0 = j * 128
                nn = cfg.win_size(j)
                ps = psC.tile([128, NCOL], F32, tag="mm", name="psn")
                for k in range(kch):
                    nc.tensor.matmul(ps[:nn, :], lh[:, koff + k, :nn],
                                     wg_sb[li][:, k, :],
                                     start=(k == 0), stop=(k == kch - 1))
                tb = npool.tile([128, ROW], BF16, tag="tb", name="tb")
                nc.vector.tensor_copy(tb[:nn, :NCOL], ps[:nn, :])
                if j < cfg.WA:
                    nc.sync.dma_start(
                        tbl_inA[li][n0:n0 + nn, :NCOL], tb[:nn, :NCOL])
                else:
                    m0 = n0 - cfg.nA
                    nc.sync.dma_start(
                        tbl_inB[li][m0:m0 + nn, :NCOL], tb[:nn, :NCOL])

            def emit_ag(li, half):
                if half == 0:
                    nc.gpsimd.collective_compute(
                        "AllGather", ALU.bypass, replica_groups=rep,
                        ins=[tbl_inA[li].opt()], outs=[tbl_agA[li].opt()])
                else:
                    nc.gpsimd.collective_compute(
                        "AllGather", ALU.bypass, replica_groups=rep,
                        ins=[tbl_inB[li].opt()], outs=[tbl_agB[li].opt()])

            def emit_ad(li):
                L = Ls[li]
                H = L["H"]
                ad_all = spool.tile([128, W, MAXH], BF16, tag="ad_all",
                                    name="ad_all", bufs=2)
                ad_tiles[li] = ad_all
                nc.vector.memset(ad_all[:], 0.0)
                ad_fA = tbl_inA[li][:, L["ad_off"]:L["ad_off"] + H]
                nc.sync.dma_start(
                    ad_all[:, :cfg.WA, :H],
                    ad_fA.rearrange("(w q) h -> q w h", q=128))
                full_b = cfg.nB // 128
                if full_b:
                    ad_fB = tbl_inB[li][:full_b * 128,
                                        L["ad_off"]:L["ad_off"] + H]
                    nc.sync.dma_start(
                        ad_all[:, cfg.WA:cfg.WA + full_b, :H],
                        ad_fB.rearrange("(w q) h -> q w h", q=128))
                if cfg.nB % 128:
                    rem = cfg.nB - full_b * 128
                    nc.sync.dma_start(
                        ad_all[:rem, cfg.WA + full_b, :H],
                        tbl_inB[li][full_b * 128:,
                                    L["ad_off"]:L["ad_off"] + H])

            # ===== MLP (node-sharded), fused with layer-0 node phase =====
            CH = 512
            nch = math.ceil(n_loc / CH)
            for j in range(nch):
                n0 = j * CH
                nn = min(CH, n_loc - n0)
                xt = npool.tile([128, 2, CH], BF16, tag="xt")
                for k in range(2):
                    nc.sync.dma_start(xt[:, k, :nn],
                                      xT[k * 128:(k + 1) * 128, n0:n0 + nn])
                ps = psC.tile([128, CH], F32, tag="mm")
                for k in range(2):
                    nc.tensor.matmul(ps[:, :nn], wm1[:, k, :], xt[:, k, :nn],
                                     start=(k == 0), stop=(k == 1))
                h1 = npool.tile([128, CH], BF16, tag="h1")
                nc.scalar.activation(h1[:, :nn], ps[:, :nn], AF.Relu,
                                     bias=b1s[:, 0:1])
                ps2 = psC.tile([128, CH], F32, tag="mm")
                nc.tensor.matmul(ps2[:, :nn], wm2[:, :], h1[:, :nn],
                                 start=True, stop=True)
                h2 = npool.tile([128, 4, 128], BF16, tag="h2")
                nc.scalar.activation(h2.rearrange("p a b -> p (a b)")[:, :nn],
                                     ps2[:, :nn], AF.Relu, bias=b2s[:, 0:1])
                for w in range(4 * j, min(4 * j + 4, W)):
                    node_chunk(0, w, h2, koff=w - 4 * j)
                    if w == cfg.WA - 1:
                        emit_ag(0, 0)

            emit_ag(0, 1)
            emit_ad(0)

            # ===== layers: flat window stream with lo-gather lookahead =====
            LOOK = getattr(cfg, "look", 10)
            GMAX = 8  # tiles per dma_gather (>1024 idxs crashes HW)
            TLOMAX = max(p.T_lo)
            THIMAX = max(p.T_hi)
            GB = max(L["row"] for L in Ls)
            seq = [(li, t) for li in range(3) for t in range(W)]
            glo_tiles = {}
            ghi_tiles = {}
            qrr = [0]  # SWDGE queue round-robin state

            def next_q():
                q = qrr[0]
                qrr[0] = (q + 1) % cfg.NQ
                return q

            # one-time memset of gather pools: with -1-truncated idx padding,
            # unwritten slots must hold bounded values (exp() stays finite)
            for _ in range(getattr(cfg, "lo_bufs", 7)):
                z = glopool.tile([128, TLOMAX * GB], BF16, tag="glo",
                                 name="glo")
                nc.vector.memset(z[:], 0.0)
            for _ in range(getattr(cfg, "hi_bufs", 3)):
                z = ghipool.tile([128, (THIMAX + 1) * GB], BF16, tag="ghi",
                                 name="ghi")
                nc.vector.memset(z[:], 0.0)

            def emit_lo(li, t):
                ROW = Ls[li]["row"]
                Tlo = p.T_lo[t]
                raw = glopool.tile([128, TLOMAX * GB], BF16, tag="glo",
                                   name="glo")
                glo = raw[:, :Tlo * ROW].rearrange("p (a b) -> p a b", b=ROW)
                glo_tiles[(li, t)] = glo
                for q0 in range(0, Tlo, GMAX):
                    q = min(GMAX, Tlo - q0)
                    nc.gpsimd.dma_gather(
                        glo[:, q0:q0 + q, :], tbl_agA[li][:, :],
                        idx_lo[:, p.win_lo_off[t] + q0 * 8:
                               p.win_lo_off[t] + (q0 + q) * 8],
                        q * 128, q * 128, ROW, queue_num=next_q())

            def emit_hi(li, t):
                L = Ls[li]
                ROW = L["row"]
                NCOL = L["HC"] + 2 * L["H"]
                Thi = p.T_hi[t]
                nn = cfg.win_size(t)
                raw = ghipool.tile([128, (THIMAX + 1) * GB], BF16, tag="ghi",
                                   name="ghi")
                ghi = raw[:, :(Thi + 1) * ROW].rearrange("p (a b) -> p a b",
                                                         b=ROW)
                ghi_tiles[(li, t)] = ghi
                # self-loop rows: direct sequential load, no gather
                if t < cfg.WA:
                    nc.sync.dma_start(
                        ghi[:nn, Thi, :NCOL],
                        tbl_inA[li][t * 128:t * 128 + nn, :NCOL])
                else:
                    m0 = t * 128 - cfg.nA
                    nc.sync.dma_start(
                        ghi[:nn, Thi, :NCOL],
                        tbl_inB[li][m0:m0 + nn, :NCOL])
                for q0 in range(0, Thi, GMAX):
                    q = min(GMAX, Thi - q0)
                    nc.gpsimd.dma_gather(
                        ghi[:, q0:q0 + q, :], tbl_agB[li][:, :],
                        idx_hi[:, p.win_hi_off[t] + q0 * 8:
                               p.win_hi_off[t] + (q0 + q) * 8],
                        q * 128, q * 128, ROW, queue_num=next_q())

            def compute(li, t):
                L = Ls[li]
                H, Cc, HC, ROW = L["H"], L["C"], L["HC"], L["row"]
                NCOL = HC + 2 * H
                ad_all = ad_tiles[li]
                Tlo, Thi = p.T_lo[t], p.T_hi[t]
                T = Tlo + Thi + 1
                nn = cfg.win_size(t)
                to = p.win_tile_off[t]
                glo = glo_tiles.pop((li, t))
                ghi = ghi_tiles.pop((li, t))

                oT = opool.tile([128, T, 128], BF16, tag="oT", name="oT")
                oD = opool.tile([128, T, 128], BF16, tag="oD", name="oD")
                for k0 in range(0, T, cfg.K_FUSE):
                    K = min(cfg.K_FUSE, T - k0)
                    nc.vector.tensor_tensor(
                        oT[:, k0:k0 + K, :],
                        iota_row.unsqueeze(1).broadcast_to([128, K, 128]),
                        dstcol[:, to + k0:to + k0 + K]
                            .unsqueeze(2).broadcast_to([128, K, 128]),
                        ALU.is_equal)
                    ptd = psC.tile([128, cfg.K_FUSE, 128], BF16, tag="ptd",
                                   name="ptd", bufs=2)
                    for i in range(K):
                        nc.tensor.transpose(ptd[:, i, :],
                                            oT[:, k0 + i, :], ident[:, :])
                    nc.scalar.activation(oD[:, k0:k0 + K, :],
                                         ptd[:, :K, :], AF.Copy)

                ps_ad = psB.tile([128, MAXT * MAXH], F32, tag="ps_ad")
                for i in range(T):
                    nc.tensor.matmul(ps_ad[:, i * H:(i + 1) * H],
                                     oD[:, i, :], ad_all[:, t, :H],
                                     start=True, stop=True)
                e_sb = spool.tile([128, MAXT * MAXH], F32, tag="e_sb")
                nc.vector.tensor_tensor(
                    e_sb[:, :Tlo * H], ps_ad[:, :Tlo * H],
                    glo[:, 0:Tlo, L["as_off"]:L["as_off"] + H],
                    ALU.add)
                nc.vector.tensor_tensor(
                    e_sb[:, Tlo * H:T * H], ps_ad[:, Tlo * H:T * H],
                    ghi[:, 0:Thi + 1, L["as_off"]:L["as_off"] + H],
                    ALU.add)
                ex1 = spool.tile([128, MAXT * MAXH], F32, tag="ex1")
                nc.scalar.activation(ex1[:, :T * H], e_sb[:, :T * H], AF.Exp)
                ex2 = spool.tile([128, MAXT * MAXH], F32, tag="ex2")
                nc.scalar.activation(ex2[:, :T * H], e_sb[:, :T * H], AF.Exp,
                                     scale=NEG_SLOPE)
                msg = mpool.tile([128, T, HC + H], BF16, tag="msg")
                nc.vector.tensor_tensor(
                    msg[:, 0:T, HC:HC + H],
                    ex1[:, :T * H], ex2[:, :T * H], ALU.max)
                # h' is (c,h)-interleaved: the per-head weight broadcasts on
                # the MIDDLE dim (c) -> DVE 2x mode stays active
                for k0 in range(0, Tlo, cfg.K_FUSE):
                    K = min(cfg.K_FUSE, Tlo - k0)
                    nc.vector.tensor_tensor(
                        msg[:, k0:k0 + K, 0:HC],
                        glo[:, k0:k0 + K, 0:HC],
                        msg[:, k0:k0 + K, HC:HC + H]
                            .unsqueeze(2).broadcast_to([128, K, Cc, H]),
                        ALU.mult)
                for k0 in range(0, Thi + 1, cfg.K_FUSE):
                    K = min(cfg.K_FUSE, Thi + 1 - k0)
                    nc.vector.tensor_tensor(
                        msg[:, Tlo + k0:Tlo + k0 + K, 0:HC],
                        ghi[:, k0:k0 + K, 0:HC],
                        msg[:, Tlo + k0:Tlo + k0 + K, HC:HC + H]
                            .unsqueeze(2).broadcast_to([128, K, Cc, H]),
                        ALU.mult)
                ps_w = psA.tile([128, HC + H], F32, tag="ps_w")
                for i in range(T):
                    nc.tensor.matmul(ps_w[:, :], oT[:, i, :],
                                     msg[:, i, :],
                                     start=(i == 0), stop=(i == T - 1))
                rcp = spool.tile([128, MAXH], F32, tag="rcp")
                nc.vector.reciprocal(rcp[:, :H], ps_w[:, HC:HC + H])
                if li == 1:
                    nc.vector.tensor_scalar_mul(rcp[:, :H], rcp[:, :H], 0.5)
                y = spool.tile([128, MAXHC], F32, tag="y")
                nc.vector.tensor_tensor(
                    y[:, :HC], ps_w[:, :HC],
                    rcp[:, :H].unsqueeze(1).broadcast_to([128, Cc, H]),
                    ALU.mult)
                if li == 1:
                    # head mean over the interleaved (c,h) pairs
                    yv = y[:, :HC].rearrange("p (c h) -> p c h", h=H)
                    ym = spool.tile([128, MAXHC], F32, tag="ym")
                    nc.vector.tensor_tensor(ym[:, :Cc], yv[:, :, 0],
                                            yv[:, :, 1], ALU.add)
                    ycols = Cc
                    ysrc = ym
                else:
                    ycols = HC
                    ysrc = y
                nc.vector.tensor_tensor(
                    y[:, :ycols], ysrc[:, :ycols],
                    bg_sb[li][:, :ycols], ALU.add)
                if li < 2:
                    e1 = spool.tile([128, MAXHC], F32, tag="elu1")
                    nc.scalar.activation(e1[:, :ycols], y[:, :ycols], AF.Exp)
                    nc.scalar.activation(e1[:, :ycols], e1[:, :ycols],
                                         AF.Relu, scale=-1.0, bias=1.0)
                    nc.scalar.activation(y[:, :ycols], y[:, :ycols], AF.Relu)
                    yb = spool.tile([128, MAXHC], BF16, tag="yb")
                    nc.vector.tensor_tensor(yb[:, :ycols], y[:, :ycols],
                                            e1[:, :ycols], ALU.subtract)
                    pts = npool.tile([128, ycols // 128, 128], BF16,
                                     tag="pts", name="pts")
                    for k in range(ycols // 128):
                        pt = psC.tile([128, 128], BF16, tag="mm")
                        nc.tensor.transpose(pt[:, :],
                                            yb[:, k * 128:(k + 1) * 128],
                                            ident[:, :])
                        nc.vector.tensor_copy(pts[:, k, :], pt[:, :])
                    node_chunk(li + 1, t, pts)
                    if t == cfg.WA - 1:
                        emit_ag(li + 1, 0)
                    if t == W - 1:
                        emit_ag(li + 1, 1)
                        emit_ad(li + 1)
                else:
                    nc.sync.dma_start(out_d[t * 128:t * 128 + nn, :],
                                      y[:nn, :ycols])

            for s0 in range(min(LOOK, len(seq))):
                emit_lo(*seq[s0])
            for step, (li, t) in enumerate(seq):
                if step + LOOK < len(seq):
                    emit_lo(*seq[step + LOOK])
                emit_hi(li, t)
                compute(li, t)
    return nc, out_d


# ---------------------------------------------------------------- runner

def make_inmaps(inputs, cfg: Cfg, p: Plan):
    wts = prep_weights(inputs, cfg)
    x = np.asarray(inputs["x"], np.float32)
    xT = np.ascontiguousarray(x.T).astype(bf)
    n_loc = cfg.n_loc
    in_maps = []
    for c in range(cfg.n_cores):
        m = dict(wts)
        m["xT"] = np.ascontiguousarray(
            xT[:, c * n_loc:(c + 1) * n_loc][:, p.perm[c]])
        m["idx_lo"] = p.idx_lo[c]
        m["idx_hi"] = p.idx_hi[c]
        m["dstcol"] = p.dstloc_col[c]
        in_maps.append(m)
    return in_maps


def build_program(cfg: Cfg, p: Plan, debug=False):
    nc = bacc.Bacc("TRN2", target_bir_lowering=False, debug=debug,
                   num_devices=cfg.n_cores, num_swdge_queues=cfg.NQ)
    build(nc, cfg, p)
    nc.compile()
    return nc


# ------------------------------------------------------------- entry point

_CACHE = {}


def kernel(**inputs):
    import numpy as _np
    from concourse.bass_utils import run_bass_kernel_spmd

    cfg = Cfg()
    ei = _np.asarray(inputs["edge_index"])
    key = hash(ei.tobytes())
    if key not in _CACHE:
        p = plan_edges(ei, cfg)
        nc = build_program(cfg, p, debug=False)
        _CACHE[key] = (p, nc)
    p, nc = _CACHE[key]
    in_maps = make_inmaps(inputs, cfg, p)
    res = run_bass_kernel_spmd(nc, in_maps, list(range(cfg.n_cores)))
    n_loc = cfg.n_loc
    out = _np.empty((cfg.N, cfg.OUT), _np.float32)
    for c in range(cfg.n_cores):
        out[c * n_loc + _np.asarray(p.perm[c])] = \
            res.results[c]["out"].astype(_np.float32)
    return out



# revision 52
# speedup vs baseline: 1.2026x; 1.0436x over previous
"""Trainium2 Bass kernel: 3-layer GAT message passing, 8-core SPMD.

Dst-sharded edge phase (dma_gather + one-hot scatter matmuls into PSUM;
self-loop rows bulk-loaded, bypassing gather descriptors), node-sharded
table build interleaved into the previous edge phase, split AllGathers.
"""
"""GAT message-passing kernel for TRN2, 8-core SPMD.

Per layer:
  node phase (node-sharded): table rows [h' | alpha_s | alpha_d | pad] for own
  nodes via PE matmuls with host-packed rhs [W | U_s | U_d].
  AllGather table -> full gather table in every core's DRAM.
  edge phase (dst-sharded): per dst-window of 128 nodes, dma_gather rows for
  the window's edges (src-indexed, int16 lo/hi split), build one-hot
  scatter/expand matrices on DVE, expand alpha_d via PE matmul,
  w = max(exp(e), exp(slope*e)) == exp(leaky_relu(e)), weighted messages
  scatter-accumulated into PSUM via one-hot matmul, normalized by the
  softmax denominator (segment max subtraction dropped; shift-invariant).
"""

import math
from dataclasses import dataclass, field

import numpy as np
import ml_dtypes

import concourse.bass as bass
import concourse.bacc as bacc
import concourse.mybir as mybir
import concourse.tile as tile
from concourse import library_config

F32 = mybir.dt.float32
BF16 = mybir.dt.bfloat16
I16 = mybir.dt.int16
AF = mybir.ActivationFunctionType
ALU = mybir.AluOpType
NEG_SLOPE = 0.2

bf = ml_dtypes.bfloat16


@dataclass
class Cfg:
    N: int = 50000
    n_cores: int = 8
    IN: int = 256
    HID: int = 128
    OUT: int = 128
    TSPLIT: int = 32768
    K_FUSE: int = 8
    NQ: int = 4  # SWDGE queues: round-robin descgen across Q7 core pairs

    @property
    def n_loc(self):
        return self.N // self.n_cores

    @property
    def n_win(self):
        return (self.n_loc + 127) // 128

    def win_size(self, t):
        return min(128, self.n_loc - t * 128)

    @property
    def WA(self):
        return 26

    @property
    def nA(self):
        return min(self.WA * 128, self.n_loc)

    @property
    def nB(self):
        return self.n_loc - self.nA

    # table pieces: each table's AllGather is split so the LAST piece is
    # small -- its wire time is the only part not hidden behind the
    # producer's remaining window projections (gathers wait on ALL pieces,
    # but earlier pieces finish during later writes).
    @property
    def WA1(self):
        return 13          # windows in piece A1 (A2 = WA - WA1)

    @property
    def WB1(self):
        return 17          # windows in piece B1 (B2 = rest)

    @property
    def nA1(self):
        return self.WA1 * 128

    @property
    def nA2(self):
        return self.nA - self.nA1

    @property
    def nB1(self):
        return self.WB1 * 128

    @property
    def nB2(self):
        return self.nB - self.nB1

    @property
    def layers(self):
        hid, out = self.HID, self.OUT
        ls = []
        for (H, C, in_ch) in ((4, hid // 2, hid), (2, out, 2 * hid), (1, out, out)):
            HC = H * C
            row = HC + 2 * H
            row_pad = ((row * 2 + 255) // 256) * 256 // 2
            ls.append(dict(H=H, C=C, in_ch=in_ch, HC=HC, row=row_pad,
                           as_off=HC, ad_off=HC + H))
        return ls


# ---------------------------------------------------------------- host plan

@dataclass
class Plan:
    T_lo: list
    T_hi: list
    idx_lo: list
    idx_hi: list
    dstloc_col: list
    dstloc_row: list
    NT: int = 0
    cols_lo: int = 0
    cols_hi: int = 0
    win_tile_off: list = field(default_factory=list)
    win_lo_off: list = field(default_factory=list)
    win_hi_off: list = field(default_factory=list)


def plan_edges(edge_index, cfg: Cfg) -> Plan:
    """Balanced plan: per core, pack local dst nodes into windows so that
    per-window (lo, hi) edge sums are near-uniform across windows AND cores
    (kills ceil + max-over-core tile padding).  Nodes with original local
    index < nA go to windows [0, WA) (table A), the rest to [WA, W) (B) --
    membership is decided by original index so lo/hi degrees are fixed
    before packing (no circularity).  perm[c][p] = original local node at
    permuted position p; inputs (xT) are host-permuted, outputs unpermuted
    in kernel()."""
    N, C = cfg.N, cfg.n_cores
    n_loc = cfg.n_loc
    W = cfg.n_win
    src = np.asarray(edge_index[0], np.int64)      # self loops handled by a
    dst = np.asarray(edge_index[1], np.int64)      # direct per-window load
    s_core = src // n_loc
    s_loc = src % n_loc
    lo_edge = s_loc < cfg.nA                       # src in table A?
    d_core = dst // n_loc
    d_loc = dst % n_loc

    # per-node lo/hi in-degree
    lo_deg = np.zeros(N, np.int64)
    hi_deg = np.zeros(N, np.int64)
    np.add.at(lo_deg, dst[lo_edge], 1)
    np.add.at(hi_deg, dst[~lo_edge], 1)

    # per-core packing
    perm = []        # perm[c][p] = original local idx at permuted pos p
    win_of = np.zeros(N, np.int64)
    slot_of = np.zeros(N, np.int64)
    for c in range(C):
        ld = lo_deg[c * n_loc:(c + 1) * n_loc]
        hd = hi_deg[c * n_loc:(c + 1) * n_loc]
        pc = np.zeros(n_loc, np.int64)
        import os
        identity = bool(int(os.environ.get("PLAN_IDENTITY", "0")))
        for half, (wins, nodes) in enumerate((
                (np.arange(cfg.WA), np.arange(cfg.nA)),
                (np.arange(cfg.WA, W), np.arange(cfg.nA, n_loc)))):
            caps = np.array([cfg.win_size(w) for w in wins], np.int64)
            lt = np.zeros(len(wins))
            ht = np.zeros(len(wins))
            space = caps.copy()
            # per-window targets proportional to window size
            lo_t = ld[nodes].sum() * caps / caps.sum()
            hi_t = hd[nodes].sum() * caps / caps.sum()
            order = (nodes if identity else
                     nodes[np.argsort(-(ld[nodes] + hd[nodes]), kind="stable")])
            fill_ptr = 0
            for nloc_i in order:
                l, h = ld[nloc_i], hd[nloc_i]
                if identity:
                    if space[fill_ptr] <= 0:
                        fill_ptr += 1
                    wi = fill_ptr
                else:
                    score = np.maximum((lt + l) - lo_t, (ht + h) - hi_t)
                    score[space <= 0] = 1 << 30
                    wi = int(np.argmin(score))
                lt[wi] += l
                ht[wi] += h
                space[wi] -= 1
                w = wins[wi]
                slot = cfg.win_size(w) - space[wi] - 1
                win_of[c * n_loc + nloc_i] = w
                slot_of[c * n_loc + nloc_i] = slot
                pc[(w * 128 if w < cfg.WA else
                    cfg.nA + (w - cfg.WA) * 128) + slot] = nloc_i
        perm.append(pc)

    # permuted position of each node within its owner's table
    pos = np.zeros(N, np.int64)
    for c in range(C):
        pc = perm[c]
        pos[c * n_loc + pc] = np.arange(n_loc)
    idx_a = s_core * cfg.nA + pos[src]             # valid when lo_edge
    idx_b = s_core * cfg.nB + (pos[src] - cfg.nA)  # valid when ~lo_edge

    win = win_of[dst]
    dloc = slot_of[dst]
    core = d_core

    buckets = {}
    for c in range(C):
        m_c = core == c
        for t in range(W):
            m = m_c & (win == t)
            ml = m & lo_edge
            mh = m & ~lo_edge
            sl, dll = idx_a[ml], dloc[ml]
            o = np.argsort(sl, kind="stable")
            buckets[(c, t, 0)] = (sl[o], dll[o])
            sh, dlh = idx_b[mh], dloc[mh]
            o = np.argsort(sh, kind="stable")
            buckets[(c, t, 1)] = (sh[o], dlh[o])

    T_lo = [max(math.ceil(len(buckets[(c, t, 0)][0]) / 128) for c in range(C))
            for t in range(W)]
    T_hi = [max(math.ceil(len(buckets[(c, t, 1)][0]) / 128) for c in range(C))
            for t in range(W)]

    NT = sum(T_lo) + sum(T_hi) + W  # +1 self tile per window
    p = Plan(T_lo=T_lo, T_hi=T_hi, idx_lo=[], idx_hi=[], dstloc_col=[],
             dstloc_row=[], NT=NT, cols_lo=max(sum(T_lo) * 8, 8),
             cols_hi=max(sum(T_hi) * 8, 8))
    p.perm = perm
    off = olo = ohi = 0
    for t in range(W):
        p.win_tile_off.append(off)
        p.win_lo_off.append(olo)
        p.win_hi_off.append(ohi)
        off += T_lo[t] + T_hi[t] + 1
        olo += T_lo[t] * 8
        ohi += T_hi[t] * 8

    for c in range(C):
        ilo = np.zeros((128, p.cols_lo), np.int16)
        ihi = np.zeros((128, p.cols_hi), np.int16)
        dcol = np.full((128, NT), -1.0, np.float32)
        for t in range(W):
            for half, (idx_arr, col_off, Tn) in enumerate(
                ((ilo, p.win_lo_off[t], T_lo[t]),
                 (ihi, p.win_hi_off[t], T_hi[t]))):
                if Tn == 0:
                    continue
                s_, dl = buckets[(c, t, half)]
                n = Tn * 128
                # pad with 0 (gathers row 0): -1 runtime truncation desyncs
                # the decode-side SWDGE ring booking from the Q7's writes
                si = np.zeros(n, np.int64)
                si[:len(s_)] = s_
                dli = np.full(n, -1.0, np.float32)
                dli[:len(dl)] = dl
                blk = si.astype(np.int16).reshape(Tn * 8, 16).T
                idx_arr[:, col_off:col_off + Tn * 8] = np.tile(blk, (8, 1))
                tb = p.win_tile_off[t] + (0 if half == 0 else T_lo[t])
                dcol[:, tb:tb + Tn] = dli.reshape(Tn, 128).T
            ts = p.win_tile_off[t] + T_lo[t] + T_hi[t]
            nn_w = min(128, n_loc - t * 128)
            selfcol = np.full(128, -1.0, np.float32)
            selfcol[:nn_w] = np.arange(nn_w)
            dcol[:, ts] = selfcol
        p.idx_lo.append(ilo)
        p.idx_hi.append(ihi)
        p.dstloc_col.append(dcol.astype(bf))
        p.dstloc_row.append(dcol.T.reshape(1, NT * 128).astype(bf))
    return p


# ------------------------------------------------------------- host weights

def prep_weights(inp, cfg: Cfg):
    """Pack per-layer projection weights.  h' columns are stored
    (c, h)-interleaved (head innermost) so the per-head message weighting
    broadcasts on a MIDDLE dim -- DVE runs 2x faster than with an
    innermost-dim broadcast."""
    out = {}
    for li, (wk, ak, dk) in enumerate(
            (("g1_W", "g1_as", "g1_ad"), ("g2_W", "g2_as", "g2_ad"),
             ("g3_W", "g3_as", "g3_ad"))):
        L = cfg.layers[li]
        Wm = np.asarray(inp[wk], np.float32)
        if li == 1:
            # layer-1 input = layer-0 output, which we emit (c,h)-interleaved
            L0 = cfg.layers[0]
            j = np.arange(L0["HC"])
            perm_in = (j % L0["H"]) * L0["C"] + j // L0["H"]
            Wm = Wm[perm_in]
        a_s = np.asarray(inp[ak], np.float32)
        a_d = np.asarray(inp[dk], np.float32)
        H, Cc = L["H"], L["C"]
        U_s = np.stack([Wm[:, h * Cc:(h + 1) * Cc] @ a_s[h] for h in range(H)], 1)
        U_d = np.stack([Wm[:, h * Cc:(h + 1) * Cc] @ a_d[h] for h in range(H)], 1)
        # (h,c) -> (c,h) column interleave
        Wm_i = np.ascontiguousarray(
            Wm.reshape(-1, H, Cc).transpose(0, 2, 1).reshape(Wm.shape))
        out[f"WG{li+1}"] = np.concatenate([Wm_i, U_s, U_d], 1).astype(bf)
        bias = np.asarray(inp[f"g{li+1}_b"], np.float32)
        if li == 0:
            bias = np.ascontiguousarray(
                bias.reshape(H, Cc).T.reshape(-1))
        out[f"bG{li+1}"] = np.ascontiguousarray(np.broadcast_to(
            bias[None, :], (128, len(bias))))
    out["Wm1"] = np.asarray(inp["W1"], np.float32).astype(bf)
    out["Wm2"] = np.asarray(inp["W2"], np.float32).astype(bf)
    out["b1c"] = np.ascontiguousarray(np.asarray(inp["b1"], np.float32)[:, None])
    out["b2c"] = np.ascontiguousarray(np.asarray(inp["b2"], np.float32)[:, None])
    out["iota_row"] = np.ascontiguousarray(
        np.broadcast_to(np.arange(128, dtype=np.float32), (128, 128))).astype(bf)
    out["iota_col"] = np.arange(128, dtype=np.float32)[:, None].astype(bf)
    out["ident"] = np.eye(128, dtype=np.float32).astype(bf)
    return out


# ---------------------------------------------------------------- builder

def build(nc, cfg: Cfg, p: Plan, dbg=False):
    W = cfg.n_win
    n_loc = cfg.n_loc
    Ls = cfg.layers
    MAXT = max(p.T_lo[t] + p.T_hi[t] for t in range(W)) + 1
    MAXHC = max(L["HC"] for L in Ls)
    MAXH = max(L["H"] for L in Ls)

    def din(name, shape, dt):
        return nc.dram_tensor(name, list(shape), dt, kind="ExternalInput")

    xT = din("xT", (cfg.IN, n_loc), BF16)
    Wm1 = din("Wm1", (cfg.IN, cfg.HID), BF16)
    Wm2 = din("Wm2", (cfg.HID, cfg.HID), BF16)
    b1c = din("b1c", (cfg.HID, 1), F32)
    b2c = din("b2c", (cfg.HID, 1), F32)
    WG = [din(f"WG{i+1}", (Ls[i]["in_ch"], Ls[i]["HC"] + 2 * Ls[i]["H"]), BF16)
          for i in range(3)]
    bG = [din(f"bG{i+1}", (128, Ls[i]["HC"] if i == 0 else Ls[i]["C"]), F32)
          for i in range(3)]
    iota_row_d = din("iota_row", (128, 128), BF16)
    iota_col_d = din("iota_col", (128, 1), BF16)
    ident_d = din("ident", (128, 128), BF16)
    idx_lo_d = din("idx_lo", (128, p.cols_lo), I16)
    idx_hi_d = din("idx_hi", (128, p.cols_hi), I16)
    dstcol_d = din("dstcol", (128, p.NT), BF16)
    out_d = nc.dram_tensor("out", [n_loc, cfg.OUT], F32, kind="ExternalOutput")
    dbg_d = {}
    if dbg:
        L0 = Ls[0]
        T0 = p.T_lo[0] + p.T_hi[0]
        for nm, shape in (("d_piece0", (cfg.HID, n_loc)),
                          ("d_tin0", (n_loc, L0["row"])),
                          ("d_tag0", (cfg.N, L0["row"])),
                          ("d_g0", (128, T0 * L0["row"])),
                          ("d_oT0", (128, T0 * 128)),
                          ("d_oD0", (128, T0 * 128)),
                          ("d_esb0", (128, T0 * L0["H"])),
                          ("d_msg0", (128, T0 * (L0["HC"] + L0["H"]))),
                          ("d_psw0", (128, L0["HC"] + L0["H"])),
                          ("d_ad0", (128, L0["H"]))):
            dt = BF16 if nm in ("d_tin0", "d_tag0", "d_g0", "d_oT0", "d_oD0",
                                "d_msg0", "d_piece0", "d_ad0") else F32
            dbg_d[nm] = nc.dram_tensor(nm, list(shape), dt, kind="ExternalOutput")

    rep = [list(range(cfg.n_cores))]

    with tile.TileContext(nc) as tc:
        with (
            tc.tile_pool(name="const", bufs=1) as cpool,
            tc.tile_pool(name="glo", bufs=getattr(cfg, "lo_bufs", 8)) as glopool,
            tc.tile_pool(name="ghi", bufs=getattr(cfg, "hi_bufs", 4)) as ghipool,
            tc.tile_pool(name="oh", bufs=3) as opool,
            tc.tile_pool(name="msg", bufs=3) as mpool,
            tc.tile_pool(name="small", bufs=4) as spool,
            tc.tile_pool(name="nodes", bufs=3) as npool,
            tc.tile_pool(name="psA", bufs=getattr(cfg, "psA_bufs", 2), space="PSUM") as psA,
            tc.tile_pool(name="psB", bufs=getattr(cfg, "psB_bufs", 2), space="PSUM") as psB,
            tc.tile_pool(name="psC", bufs=2, space="PSUM") as psC,
            tc.tile_pool(name="dram", bufs=1, space="DRAM") as dpool,
        ):
            nc.gpsimd.load_library(library_config.mlp)

            def load_const(handle, shape, dtp, tag):
                t = cpool.tile(list(shape), dtp, tag=tag, name=tag)
                nc.sync.dma_start(t[:], handle[:])
                return t

            iota_row = load_const(iota_row_d, (128, 128), BF16, "iota_row")
            iota_col = load_const(iota_col_d, (128, 1), BF16, "iota_col")
            ident = load_const(ident_d, (128, 128), BF16, "ident")
            idx_lo = load_const(idx_lo_d, (128, p.cols_lo), I16, "idx_lo")
            idx_hi = load_const(idx_hi_d, (128, p.cols_hi), I16, "idx_hi")
            dstcol = load_const(dstcol_d, (128, p.NT), BF16, "dstcol")
            wg_sb = []
            for i in range(3):
                L = Ls[i]
                kch = L["in_ch"] // 128
                t = cpool.tile([128, kch, L["HC"] + 2 * L["H"]], BF16, tag=f"wg{i}",
                               name=f"wg{i}")
                for k in range(kch):
                    nc.sync.dma_start(t[:, k, :], WG[i][k * 128:(k + 1) * 128, :])
                wg_sb.append(t)
            bg_sb = [load_const(bG[i], (128, Ls[i]["HC"] if i == 0 else Ls[i]["C"]),
                                F32, f"bg{i}") for i in range(3)]
            wm1 = cpool.tile([128, 2, cfg.HID], BF16, tag="wm1")
            for k in range(2):
                nc.sync.dma_start(wm1[:, k, :], Wm1[k * 128:(k + 1) * 128, :])
            wm2 = load_const(Wm2, (cfg.HID, cfg.HID), BF16, "wm2")
            b1s = load_const(b1c, (cfg.HID, 1), F32, "b1s")
            b2s = load_const(b2c, (cfg.HID, 1), F32, "b2s")

            tbl_inA = [dpool.tile([cfg.nA, Ls[i]["row"]], BF16, tag=f"tinA{i}",
                                  name=f"tinA{i}")
                       for i in range(3)]
            tbl_inB = [dpool.tile([cfg.nB, Ls[i]["row"]], BF16, tag=f"tinB{i}",
                                  name=f"tinB{i}")
                       for i in range(3)]
            tbl_agA = [dpool.tile([cfg.nA * cfg.n_cores, Ls[i]["row"]], BF16,
                                  tag=f"tagA{i}", name=f"tagA{i}",
                                  addr_space="Shared")
                       for i in range(3)]
            tbl_agB = [dpool.tile([cfg.nB * cfg.n_cores, Ls[i]["row"]], BF16,
                                  tag=f"tagB{i}", name=f"tagB{i}",
                                  addr_space="Shared")
                       for i in range(3)]

            ad_tiles = {}

            def node_chunk(li, j, lh, koff=0):
                """Project SBUF-resident h columns (lh[:, koff+k, :nn]) of
                window j into table rows and store to local DRAM table."""
                L = Ls[li]
                kch = L["in_ch"] // 128
                NCOL = L["HC"] + 2 * L["H"]
                ROW = L["row"]
                n0 = j * 128
                nn = cfg.win_size(j)
                ps = psC.tile([128, NCOL], F32, tag="mm", name="psn")
                for k in range(kch):
                    nc.tensor.matmul(ps[:nn, :], lh[:, koff + k, :nn],
                                     wg_sb[li][:, k, :],
                                     start=(k == 0), stop=(k == kch - 1))
                tb = npool.tile([128, ROW], BF16, tag="tb", name="tb")
                nc.vector.tensor_copy(tb[:nn, :NCOL], ps[:nn, :])
                if j < cfg.WA:
                    nc.sync.dma_start(
                        tbl_inA[li][n0:n0 + nn, :NCOL], tb[:nn, :NCOL])
                else:
                    m0 = n0 - cfg.nA
                    nc.sync.dma_start(
                        tbl_inB[li][m0:m0 + nn, :NCOL], tb[:nn, :NCOL])

            def emit_ag(li, half):
                if half == 0:
                    nc.gpsimd.collective_compute(
                        "AllGather", ALU.bypass, replica_groups=rep,
                        ins=[tbl_inA[li].opt()], outs=[tbl_agA[li].opt()])
                else:
                    nc.gpsimd.collective_compute(
                        "AllGather", ALU.bypass, replica_groups=rep,
                        ins=[tbl_inB[li].opt()], outs=[tbl_agB[li].opt()])

            def emit_ad(li):
                L = Ls[li]
                H = L["H"]
                ad_all = spool.tile([128, W, MAXH], BF16, tag="ad_all",
                                    name="ad_all", bufs=2)
                ad_tiles[li] = ad_all
                nc.vector.memset(ad_all[:], 0.0)
                ad_fA = tbl_inA[li][:, L["ad_off"]:L["ad_off"] + H]
                nc.sync.dma_start(
                    ad_all[:, :cfg.WA, :H],
                    ad_fA.rearrange("(w q) h -> q w h", q=128))
                full_b = cfg.nB // 128
                if full_b:
                    ad_fB = tbl_inB[li][:full_b * 128,
                                        L["ad_off"]:L["ad_off"] + H]
                    nc.sync.dma_start(
                        ad_all[:, cfg.WA:cfg.WA + full_b, :H],
                        ad_fB.rearrange("(w q) h -> q w h", q=128))
                if cfg.nB % 128:
                    rem = cfg.nB - full_b * 128
                    nc.sync.dma_start(
                        ad_all[:rem, cfg.WA + full_b, :H],
                        tbl_inB[li][full_b * 128:,
                                    L["ad_off"]:L["ad_off"] + H])

            # ===== MLP (node-sharded), fused with layer-0 node phase =====
            CH = 512
            nch = math.ceil(n_loc / CH)
            for j in range(nch):
                n0 = j * CH
                nn = min(CH, n_loc - n0)
                xt = npool.tile([128, 2, CH], BF16, tag="xt")
                for k in range(2):
                    nc.sync.dma_start(xt[:, k, :nn],
                                      xT[k * 128:(k + 1) * 128, n0:n0 + nn])
                ps = psC.tile([128, CH], F32, tag="mm")
                for k in range(2):
                    nc.tensor.matmul(ps[:, :nn], wm1[:, k, :], xt[:, k, :nn],
                                     start=(k == 0), stop=(k == 1))
                h1 = npool.tile([128, CH], BF16, tag="h1")
                nc.scalar.activation(h1[:, :nn], ps[:, :nn], AF.Relu,
                                     bias=b1s[:, 0:1])
                ps2 = psC.tile([128, CH], F32, tag="mm")
                nc.tensor.matmul(ps2[:, :nn], wm2[:, :], h1[:, :nn],
                                 start=True, stop=True)
                h2 = npool.tile([128, 4, 128], BF16, tag="h2")
                nc.scalar.activation(h2.rearrange("p a b -> p (a b)")[:, :nn],
                                     ps2[:, :nn], AF.Relu, bias=b2s[:, 0:1])
                for w in range(4 * j, min(4 * j + 4, W)):
                    node_chunk(0, w, h2, koff=w - 4 * j)
                    if w == cfg.WA - 1:
                        emit_ag(0, 0)

            emit_ag(0, 1)
            emit_ad(0)

            # ===== layers: flat window stream with lo-gather lookahead =====
            LOOK = getattr(cfg, "look", 7)
            GMAX = 8  # tiles per dma_gather (>1024 idxs crashes HW)
            TLOMAX = max(p.T_lo)
            THIMAX = max(p.T_hi)
            GB = max(L["row"] for L in Ls)
            seq = [(li, t) for li in range(3) for t in range(W)]
            glo_tiles = {}
            ghi_tiles = {}
            qrr = [0]  # SWDGE queue round-robin state

            def next_q():
                q = qrr[0]
                qrr[0] = (q + 1) % cfg.NQ
                return q

            # one-time memset of gather pools: with -1-truncated idx padding,
            # unwritten slots must hold bounded values (exp() stays finite)
            for _ in range(getattr(cfg, "lo_bufs", 8)):
                z = glopool.tile([128, TLOMAX * GB], BF16, tag="glo",
                                 name="glo")
                nc.vector.memset(z[:], 0.0)
            for _ in range(getattr(cfg, "hi_bufs", 4)):
                z = ghipool.tile([128, (THIMAX + 1) * GB], BF16, tag="ghi",
                                 name="ghi")
                nc.vector.memset(z[:], 0.0)

            def emit_lo(li, t):
                ROW = Ls[li]["row"]
                Tlo = p.T_lo[t]
                raw = glopool.tile([128, TLOMAX * GB], BF16, tag="glo",
                                   name="glo")
                glo = raw[:, :Tlo * ROW].rearrange("p (a b) -> p a b", b=ROW)
                glo_tiles[(li, t)] = glo
                for q0 in range(0, Tlo, GMAX):
                    q = min(GMAX, Tlo - q0)
                    nc.gpsimd.dma_gather(
                        glo[:, q0:q0 + q, :], tbl_agA[li][:, :],
                        idx_lo[:, p.win_lo_off[t] + q0 * 8:
                               p.win_lo_off[t] + (q0 + q) * 8],
                        q * 128, q * 128, ROW, queue_num=next_q())

            def emit_hi(li, t):
                L = Ls[li]
                ROW = L["row"]
                NCOL = L["HC"] + 2 * L["H"]
                Thi = p.T_hi[t]
                nn = cfg.win_size(t)
                raw = ghipool.tile([128, (THIMAX + 1) * GB], BF16, tag="ghi",
                                   name="ghi")
                ghi = raw[:, :(Thi + 1) * ROW].rearrange("p (a b) -> p a b",
                                                         b=ROW)
                ghi_tiles[(li, t)] = ghi
                # self-loop rows: direct sequential load, no gather
                if t < cfg.WA:
                    nc.sync.dma_start(
                        ghi[:nn, Thi, :NCOL],
                        tbl_inA[li][t * 128:t * 128 + nn, :NCOL])
                else:
                    m0 = t * 128 - cfg.nA
                    nc.sync.dma_start(
                        ghi[:nn, Thi, :NCOL],
                        tbl_inB[li][m0:m0 + nn, :NCOL])
                for q0 in range(0, Thi, GMAX):
                    q = min(GMAX, Thi - q0)
                    nc.gpsimd.dma_gather(
                        ghi[:, q0:q0 + q, :], tbl_agB[li][:, :],
                        idx_hi[:, p.win_hi_off[t] + q0 * 8:
                               p.win_hi_off[t] + (q0 + q) * 8],
                        q * 128, q * 128, ROW, queue_num=next_q())

            def compute(li, t):
                L = Ls[li]
                H, Cc, HC, ROW = L["H"], L["C"], L["HC"], L["row"]
                NCOL = HC + 2 * H
                ad_all = ad_tiles[li]
                Tlo, Thi = p.T_lo[t], p.T_hi[t]
                T = Tlo + Thi + 1
                nn = cfg.win_size(t)
                to = p.win_tile_off[t]
                glo = glo_tiles.pop((li, t))
                ghi = ghi_tiles.pop((li, t))

                oT = opool.tile([128, T, 128], BF16, tag="oT", name="oT")
                oD = opool.tile([128, T, 128], BF16, tag="oD", name="oD")
                for k0 in range(0, T, cfg.K_FUSE):
                    K = min(cfg.K_FUSE, T - k0)
                    nc.vector.tensor_tensor(
                        oT[:, k0:k0 + K, :],
                        iota_row.unsqueeze(1).broadcast_to([128, K, 128]),
                        dstcol[:, to + k0:to + k0 + K]
                            .unsqueeze(2).broadcast_to([128, K, 128]),
                        ALU.is_equal)
                    ptd = psC.tile([128, cfg.K_FUSE, 128], BF16, tag="ptd",
                                   name="ptd", bufs=2)
                    for i in range(K):
                        nc.tensor.transpose(ptd[:, i, :],
                                            oT[:, k0 + i, :], ident[:, :])
                    nc.scalar.activation(oD[:, k0:k0 + K, :],
                                         ptd[:, :K, :], AF.Copy)

                ps_ad = psB.tile([128, MAXT * MAXH], F32, tag="ps_ad")
                for i in range(T):
                    nc.tensor.matmul(ps_ad[:, i * H:(i + 1) * H],
                                     oD[:, i, :], ad_all[:, t, :H],
                                     start=True, stop=True)
                e_sb = spool.tile([128, MAXT * MAXH], F32, tag="e_sb")
                nc.vector.tensor_tensor(
                    e_sb[:, :Tlo * H], ps_ad[:, :Tlo * H],
                    glo[:, 0:Tlo, L["as_off"]:L["as_off"] + H],
                    ALU.add)
                nc.vector.tensor_tensor(
                    e_sb[:, Tlo * H:T * H], ps_ad[:, Tlo * H:T * H],
                    ghi[:, 0:Thi + 1, L["as_off"]:L["as_off"] + H],
                    ALU.add)
                ex1 = spool.tile([128, MAXT * MAXH], F32, tag="ex1")
                nc.scalar.activation(ex1[:, :T * H], e_sb[:, :T * H], AF.Exp)
                ex2 = spool.tile([128, MAXT * MAXH], F32, tag="ex2")
                nc.scalar.activation(ex2[:, :T * H], e_sb[:, :T * H], AF.Exp,
                                     scale=NEG_SLOPE)
                msg = mpool.tile([128, T, HC + H], BF16, tag="msg")
                nc.vector.tensor_tensor(
                    msg[:, 0:T, HC:HC + H],
                    ex1[:, :T * H], ex2[:, :T * H], ALU.max)
                # h' is (c,h)-interleaved: the per-head weight broadcasts on
                # the MIDDLE dim (c) -> DVE 2x mode stays active
                for k0 in range(0, Tlo, cfg.K_FUSE):
                    K = min(cfg.K_FUSE, Tlo - k0)
                    nc.vector.tensor_tensor(
                        msg[:, k0:k0 + K, 0:HC],
                        glo[:, k0:k0 + K, 0:HC],
                        msg[:, k0:k0 + K, HC:HC + H]
                            .unsqueeze(2).broadcast_to([128, K, Cc, H]),
                        ALU.mult)
                for k0 in range(0, Thi + 1, cfg.K_FUSE):
                    K = min(cfg.K_FUSE, Thi + 1 - k0)
                    nc.vector.tensor_tensor(
                        msg[:, Tlo + k0:Tlo + k0 + K, 0:HC],
                        ghi[:, k0:k0 + K, 0:HC],
                        msg[:, Tlo + k0:Tlo + k0 + K, HC:HC + H]
                            .unsqueeze(2).broadcast_to([128, K, Cc, H]),
                        ALU.mult)
                ps_w = psA.tile([128, HC + H], F32, tag="ps_w")
                for i in range(T):
                    nc.tensor.matmul(ps_w[:, :], oT[:, i, :],
                                     msg[:, i, :],
                                     start=(i == 0), stop=(i == T - 1))
                rcp = spool.tile([128, MAXH], F32, tag="rcp")
                nc.vector.reciprocal(rcp[:, :H], ps_w[:, HC:HC + H])
                if li == 1:
                    nc.vector.tensor_scalar_mul(rcp[:, :H], rcp[:, :H], 0.5)
                y = spool.tile([128, MAXHC], F32, tag="y")
                nc.vector.tensor_tensor(
                    y[:, :HC], ps_w[:, :HC],
                    rcp[:, :H].unsqueeze(1).broadcast_to([128, Cc, H]),
                    ALU.mult)
                if li == 1:
                    # head mean over the interleaved (c,h) pairs
                    yv = y[:, :HC].rearrange("p (c h) -> p c h", h=H)
                    ym = spool.tile([128, MAXHC], F32, tag="ym")
                    nc.vector.tensor_tensor(ym[:, :Cc], yv[:, :, 0],
                                            yv[:, :, 1], ALU.add)
                    ycols = Cc
                    ysrc = ym
                else:
                    ycols = HC
                    ysrc = y
                nc.vector.tensor_tensor(
                    y[:, :ycols], ysrc[:, :ycols],
                    bg_sb[li][:, :ycols], ALU.add)
                if li < 2:
                    e1 = spool.tile([128, MAXHC], F32, tag="elu1")
                    nc.scalar.activation(e1[:, :ycols], y[:, :ycols], AF.Exp)
                    nc.scalar.activation(e1[:, :ycols], e1[:, :ycols],
                                         AF.Relu, scale=-1.0, bias=1.0)
                    nc.scalar.activation(y[:, :ycols], y[:, :ycols], AF.Relu)
                    yb = spool.tile([128, MAXHC], BF16, tag="yb")
                    nc.vector.tensor_tensor(yb[:, :ycols], y[:, :ycols],
                                            e1[:, :ycols], ALU.subtract)
                    pts = npool.tile([128, ycols // 128, 128], BF16,
                                     tag="pts", name="pts")
                    for k in range(ycols // 128):
                        pt = psC.tile([128, 128], BF16, tag="mm")
                        nc.tensor.transpose(pt[:, :],
                                            yb[:, k * 128:(k + 1) * 128],
                                            ident[:, :])
                        nc.vector.tensor_copy(pts[:, k, :], pt[:, :])
                    node_chunk(li + 1, t, pts)
                    if t == cfg.WA - 1:
                        emit_ag(li + 1, 0)
                    if t == W - 1:
                        emit_ag(li + 1, 1)
                        emit_ad(li + 1)
                else:
                    nc.sync.dma_start(out_d[t * 128:t * 128 + nn, :],
                                      y[:nn, :ycols])

            for s0 in range(min(LOOK, len(seq))):
                emit_lo(*seq[s0])
            for step, (li, t) in enumerate(seq):
                if step + LOOK < len(seq):
                    emit_lo(*seq[step + LOOK])
                emit_hi(li, t)
                compute(li, t)
    return nc, out_d


# ---------------------------------------------------------------- runner

def make_inmaps(inputs, cfg: Cfg, p: Plan):
    wts = prep_weights(inputs, cfg)
    x = np.asarray(inputs["x"], np.float32)
    xT = np.ascontiguousarray(x.T).astype(bf)
    n_loc = cfg.n_loc
    in_maps = []
    for c in range(cfg.n_cores):
        m = dict(wts)
        m["xT"] = np.ascontiguousarray(
            xT[:, c * n_loc:(c + 1) * n_loc][:, p.perm[c]])
        m["idx_lo"] = p.idx_lo[c]
        m["idx_hi"] = p.idx_hi[c]
        m["dstcol"] = p.dstloc_col[c]
        in_maps.append(m)
    return in_maps


def build_program(cfg: Cfg, p: Plan, debug=False):
    nc = bacc.Bacc("TRN2", target_bir_lowering=False, debug=debug,
                   num_devices=cfg.n_cores, num_swdge_queues=cfg.NQ)
    build(nc, cfg, p)
    nc.compile()
    return nc


# ------------------------------------------------------------- entry point

_CACHE = {}


def kernel(**inputs):
    import numpy as _np
    from concourse.bass_utils import run_bass_kernel_spmd

    cfg = Cfg()
    ei = _np.asarray(inputs["edge_index"])
    key = hash(ei.tobytes())
    if key not in _CACHE:
        p = plan_edges(ei, cfg)
        nc = build_program(cfg, p, debug=False)
        _CACHE[key] = (p, nc)
    p, nc = _CACHE[key]
    in_maps = make_inmaps(inputs, cfg, p)
    res = run_bass_kernel_spmd(nc, in_maps, list(range(cfg.n_cores)))
    n_loc = cfg.n_loc
    out = _np.empty((cfg.N, cfg.OUT), _np.float32)
    for c in range(cfg.n_cores):
        out[c * n_loc + _np.asarray(p.perm[c])] = \
            res.results[c]["out"].astype(_np.float32)
    return out

